# revision 1
# baseline (speedup 1.0000x reference)
"""Trainium2 Bass kernel for nn_MeshCrossAttention (mesh cross-attention + per-head MLP).

Sharding: data-parallel over batch B=16 -> 2 batches per NeuronCore, 8 cores,
no collectives. Inside each core everything runs in a "transposed chain":

  host pre-transposes activations/weights so every matmul contracts over the
  partition dim with zero on-device transposes:

    qT  [D, LQ]   = Wq @ query^T        (lhsT = WqT chunk, rhs = queryT chunk)
    kT  [D, LK]   = Wk @ key^T
    v   [LK, D]   = value @ Wv^T        (lhsT = valueT chunk, rhs = WvT)
                    stored head-interleaved [LK, H, HD+1] with a ones column
    sT  [LK, LQ]  = kT_h^T @ qT_h       (K=64 contraction, heads pair-packed
                                         into rows 0-63 / 64-127 of the PE)
    eT  [LK, LQ]  = exp(sT / 8)         (ScalarE, fused scale)
    cT  [HD+1,LQ] = v_aug^T @ eT        (row HD = softmax denominator, free)
    rT  = 1/denom                        (batched DVE reciprocal_approx_accurate)
    bc  [HD, LQ]  = ones ⊗ rT           (PE outer product K=1 -> partition bcast)
    cn  = cT * bc                        (DVE, psum operand)
    h1T [HD, LQ]  = gelu(W1T^T @ cn_cat + b1)   (bf16 out)
    out [LQ, 2HD] = h1T_pair^T @ blockdiag(W2T) (bf16, natural layout) + b2
"""
import math
import sys

import numpy as np

if "/opt/trn_rl_repo" not in sys.path:
    sys.path.insert(0, "/opt/trn_rl_repo")

import ml_dtypes  # noqa: E402

import concourse.bass as bass  # noqa: E402
import concourse.tile as tile  # noqa: E402
from concourse import bacc, mybir  # noqa: E402
from concourse.bass_utils import run_bass_kernel_spmd  # noqa: E402

F32 = mybir.dt.float32
F32R = mybir.dt.float32r
BF16 = mybir.dt.bfloat16

D, H, HD, J = 1024, 16, 64, 3
B, LQ, LK = 16, 512, 512
P = 128
N_CORES = 8
B_LOC = B // N_CORES  # 2
HG = 4                # heads per normalization group


def _emit(tc, aps, dbg=False):
    nc = tc.nc
    ctx_mgr = []

    def pool(name, bufs, space="SBUF"):
        p = tc.tile_pool(name=name, bufs=bufs, space=space)
        ctx_mgr.append(p)
        return p.__enter__()

    const = pool("const", 1)
    ain = pool("ain", 9)           # input activation chunks [128,512]
    wpool = pool("w", 2)           # weight chunks [128,1024]
    qt_pool = pool("qt", 8)
    kt_pool = pool("kt", 24)
    va_pool = pool("va", 3)        # v_aug [128, 4, 1040]
    expp = pool("expp", 3)
    ctxp = pool("ctxp", 12)        # unnormalized ctx [64, 512]
    catp = pool("catp", 2)         # ctx_cat chunks
    denp = pool("denp", 1)
    bcp = pool("bcp", 2)           # broadcast recip rows [64, 512]
    h1p = pool("h1p", 3)
    ostg = pool("ostg", 4)

    dramp = pool("dramp", 2, "DRAM")      # recip round-trip for DMA broadcast
    ps_big = pool("ps_big", 4, "PSUM")    # proj + scores [128,512]
    ps_ctx = pool("ps_ctx", 2, "PSUM")    # [65,512]
    ps_mlp = pool("ps_mlp", 1, "PSUM")    # mlp1 [64,512] + mlp2 [128,128] tags

    # ---------------- constants ----------------
    w1t_a = const.tile([P, HD], F32R, tag="w1a")        # W1T rows 0:128
    w1t_b = const.tile([HD, HD], F32R, tag="w1b")       # W1T rows 128:192
    nc.sync.dma_start(out=w1t_a[:], in_=aps["w1t"][0:P, :])
    nc.sync.dma_start(out=w1t_b[:], in_=aps["w1t"][P:J * HD, :])

    w2bd = const.tile([P, P], BF16, tag="w2bd")
    nc.sync.dma_start(out=w2bd[:], in_=aps["w2bd"][:, :])
    b2bd = const.tile([P, P], F32, tag="b2bd")
    nc.sync.dma_start(out=b2bd[:], in_=aps["b2bd"][:, :])
    bv_bc = const.tile([P, D], F32, tag="bv_bc")
    nc.sync.dma_start(out=bv_bc[:], in_=aps["bv_bc"][:, :])

    bq_sb = const.tile([P, 8], F32, tag="bq")  # col oc = bias chunk oc
    bk_sb = const.tile([P, 8], F32, tag="bk")
    nc.sync.dma_start(out=bq_sb[:], in_=aps["bq"].rearrange("(oc p) -> p oc", p=P))
    nc.sync.dma_start(out=bk_sb[:], in_=aps["bk"].rearrange("(oc p) -> p oc", p=P))
    b1_sb = const.tile([HD, 1], F32, tag="b1")
    nc.sync.dma_start(out=b1_sb[:], in_=aps["b1"].unsqueeze(1))

    def load_acts(ap_slice):
        ts = []
        for ic in range(8):
            t = ain.tile([P, 512], F32R, tag="ain")
            nc.sync.dma_start(out=t[:], in_=ap_slice[ic * P:(ic + 1) * P, :])
            ts.append(t)
        return ts

    def proj_T(w_ap, x_tiles, bias_sb, out_pool, out_tag):
        """Transposed projection: out[oc] [128, 512] = (W @ x^T) chunk + bias."""
        outs = []
        for ocb in range(2):
            pss = [ps_big.tile([P, 512], F32, tag="big", name=f"pss{_i}") for _i in range(4)]
            for ic in range(8):
                wt = wpool.tile([P, 512], F32R, tag="w")
                nc.sync.dma_start(
                    out=wt[:],
                    in_=w_ap[ic * P:(ic + 1) * P, ocb * 512:(ocb + 1) * 512])
                for o4 in range(4):
                    nc.tensor.matmul(
                        out=pss[o4][:], lhsT=(wt[:, o4 * P:(o4 + 1) * P]),
                        rhs=(x_tiles[ic][:]), start=(ic == 0), stop=(ic == 7))
            for o4 in range(4):
                oc = ocb * 4 + o4
                t = out_pool.tile([P, 512], F32R, tag=out_tag)
                nc.vector.tensor_scalar_add(t[:], pss[o4][:], bias_sb[:, oc:oc + 1])
                outs.append(t)
        return outs

    for b in range(B_LOC):
        # ================= projections =================
        qin = load_acts(aps["qt_in"][b])
        qT = proj_T(aps["wqt"], qin, bq_sb, qt_pool, "qt")

        kT = []
        for j in range(J):
            kin = load_acts(aps["kt_in"][j, b])
            kT.append(proj_T(aps["wkt"], kin, bk_sb, kt_pool, "kt"))

        # ---- V: natural, head-interleaved v_aug [128, 4, H*(HD+1)] ----
        v_aug = []
        for j in range(J):
            vin = load_acts(aps["vt_in"][j, b])
            va = va_pool.tile([P, 4, H * (HD + 1)], F32R, tag="va")
            # ones columns (all 16 at position HD within each head stripe)
            nc.sync.dma_start(
                out=va[:, :, :].rearrange("p c (h e) -> p c h e", e=HD + 1)[:, :, :, HD],
                in_=aps["ones_cols"][:, :, :])
            for half in range(2):
                pss = [ps_big.tile([P, 512], F32, tag="big", name=f"pss{_i}") for _i in range(4)]
                for ic in range(8):
                    wt = wpool.tile([P, 512], F32R, tag="w")
                    nc.sync.dma_start(
                        out=wt[:],
                        in_=aps["wvt"][ic * P:(ic + 1) * P,
                                       half * 512:(half + 1) * 512])
                    for nck in range(4):
                        nc.tensor.matmul(
                            out=pss[nck][:],
                            lhsT=(vin[ic][:, nck * P:(nck + 1) * P]),
                            rhs=(wt[:]), start=(ic == 0), stop=(ic == 7))
                for nck in range(4):
                    dst = va[:, nck, :].rearrange("p (h e) -> p h e", e=HD + 1)[
                        :, half * 8:(half + 1) * 8, 0:HD]
                    nc.vector.tensor_tensor(
                        out=dst,
                        in0=pss[nck][:].rearrange("p (h e) -> p h e", e=HD),
                        in1=bv_bc[:, half * 512:(half + 1) * 512].rearrange(
                            "p (h e) -> p h e", e=HD),
                        op=mybir.AluOpType.add)
            v_aug.append(va)

        if dbg and b == 0:
            for oc in range(8):
                nc.sync.dma_start(out=aps["dbg_qt"][oc], in_=qT[oc][:])
                nc.sync.dma_start(out=aps["dbg_kt0"][oc], in_=kT[0][oc][:])
            nc.sync.dma_start(out=aps["dbg_va0"][:, :, :], in_=v_aug[0][:])

        # ================= attention + MLP =================
        # den/rstage layout: partition 32*gi, free cols j*LQ:(j+1)*LQ
        if b == 0:
            den = denp.tile([97, J * LQ], F32, tag="den")
            rstage = denp.tile([97, J * LQ], F32, tag="rstage")
            scratch = denp.tile([97, J * LQ], F32, tag="rscratch")
            nc.vector.memset(den[:], 1.0)  # init unused lanes for reciprocal
        for hg in range(H // HG):
            heads = list(range(hg * HG, hg * HG + HG))
            ctx_tiles = {}
            for gi, h in enumerate(heads):
                ti, r0 = h // 2, (h % 2) * HD
                for j in range(J):
                    psc = ps_ctx.tile([HD + 1, LQ], F32, tag="ctx")
                    for ci in range(4):
                        pss = ps_big.tile([P, LQ], F32, tag="big")
                        nc.tensor.matmul(
                            out=pss[:],
                            lhsT=(kT[j][ti][r0:r0 + HD, ci * P:(ci + 1) * P]),
                            rhs=(qT[ti][r0:r0 + HD, :]),
                            start=True, stop=True)
                        et = expp.tile([P, LQ], F32R, tag="expp")
                        nc.scalar.activation(
                            out=et[:], in_=pss[:],
                            func=mybir.ActivationFunctionType.Exp,
                            scale=1.0 / math.sqrt(HD))
                        va_l = v_aug[j][:, ci, :].rearrange(
                            "p (h e) -> p h e", e=HD + 1)[:, h, :]
                        nc.tensor.matmul(
                            out=psc[:], lhsT=(va_l), rhs=(et[:]),
                            start=(ci == 0), stop=(ci == 3))
                        if dbg and b == 0 and h == 0 and j == 0:
                            nc.sync.dma_start(out=aps["dbg_exp"][ci], in_=et[:])
                    ct = ctxp.tile([HD, LQ], F32, tag="ctxp")
                    nc.scalar.copy(out=ct[:], in_=psc[0:HD, :])
                    nc.vector.tensor_copy(
                        out=den[32 * gi:32 * gi + 1, j * LQ:(j + 1) * LQ],
                        in_=psc[HD:HD + 1, :])
                    ctx_tiles[(j, h)] = ct
                    if dbg and b == 0 and hg == 0:
                        nc.sync.dma_start(out=aps["dbg_ctx"][gi * J + j],
                                          in_=ct[:])
            nc.vector.reciprocal_approx_accurate(
                out=rstage[:], in_=den[:], scratch=scratch[:])
            dram_r = dramp.tile([97, J * LQ], F32, tag="dram_r")
            nc.sync.dma_start(out=dram_r[:, :], in_=rstage[:])
            if dbg and b == 0 and hg == 0:
                nc.sync.dma_start(out=aps["dbg_den"][:, :], in_=den[:])
                nc.sync.dma_start(out=aps["dbg_rst"][:, :], in_=rstage[:])

            for gi, h in enumerate(heads):
                cat0 = catp.tile([P, LQ], F32R, tag="cat0")   # j0 | j1
                cat1 = catp.tile([HD, LQ], F32R, tag="cat1")  # j2
                for j in range(J):
                    bc = bcp.tile([HD, LQ], F32, tag="bc")
                    nc.sync.dma_start(
                        out=bc[:],
                        in_=dram_r[32 * gi:32 * gi + 1, j * LQ:(j + 1) * LQ]
                        .to_broadcast((HD, LQ)))
                    dst = cat0[j * HD:(j + 1) * HD, :] if j < 2 else cat1[:]
                    nc.vector.tensor_mul(dst, ctx_tiles[(j, h)][:], bc[:])
                if dbg and b == 0 and h == 0:
                    nc.sync.dma_start(out=aps["dbg_cat0"][:, :], in_=cat0[:])
                    nc.sync.dma_start(out=aps["dbg_cat1"][:, :], in_=cat1[:])
                ph1 = ps_mlp.tile([HD, LQ], F32, tag="mlp1")
                nc.tensor.matmul(out=ph1[:], lhsT=(w1t_a[:]), rhs=(cat0[:]),
                                 start=True, stop=False)
                nc.tensor.matmul(out=ph1[:], lhsT=(w1t_b[:]), rhs=(cat1[:]),
                                 start=False, stop=True)
                if h % 2 == 0:
                    h1_pair = h1p.tile([P, LQ], BF16, tag="h1")
                nc.scalar.activation(
                    out=h1_pair[(h % 2) * HD:(h % 2) * HD + HD, :], in_=ph1[:],
                    func=mybir.ActivationFunctionType.Gelu, bias=b1_sb[:])
                if h % 2 == 1:
                    hp = h // 2
                    for ncf in range(4):
                        ps2 = ps_mlp.tile([P, P], F32, tag="mlp2")
                        nc.tensor.matmul(
                            out=ps2[:], lhsT=h1_pair[:, ncf * P:(ncf + 1) * P],
                            rhs=w2bd[:], start=True, stop=True)
                        ot = ostg.tile([P, P], F32, tag="ostg")
                        nc.vector.tensor_add(ot[:], ps2[:], b2bd[:])
                        nc.sync.dma_start(
                            out=aps["out"][b, ncf * P:(ncf + 1) * P,
                                           hp * P:(hp + 1) * P],
                            in_=ot[:])

    for p in reversed(ctx_mgr):
        p.__exit__(None, None, None)


_CACHE = {}


def _build(dbg=False):
    key = ("nc", dbg)
    if key in _CACHE:
        return _CACHE[key]
    nc = bacc.Bacc("TRN2", target_bir_lowering=False, debug=False)
    shapes = {
        "qt_in": ([B_LOC, D, LQ], F32R),
        "kt_in": ([J, B_LOC, D, LK], F32R),
        "vt_in": ([J, B_LOC, D, LK], F32R),
        "wqt": ([D, D], F32R),
        "wkt": ([D, D], F32R),
        "wvt": ([D, D], F32R),
        "w1t": ([J * HD, HD], F32R),
        "ones_cols": ([P, 4, H], F32R),
        "w2bd": ([P, P], BF16),
        "b2bd": ([P, P], F32),
        "bv_bc": ([P, D], F32),
        "bq": ([D], F32),
        "bk": ([D], F32),
        "b1": ([HD], F32),
    }
    aps = {k: nc.dram_tensor(k, s, dt, kind="ExternalInput").ap()
           for k, (s, dt) in shapes.items()}
    aps["out"] = nc.dram_tensor("out", [B_LOC, LQ, D], F32,
                                kind="ExternalOutput").ap()
    if dbg:
        dbg_shapes = {
            "dbg_qt": [8, P, 512], "dbg_kt0": [8, P, 512],
            "dbg_va0": [P, 4, H * (HD + 1)], "dbg_exp": [4, P, LQ],
            "dbg_ctx": [12, HD, LQ], "dbg_den": [97, J * LQ],
            "dbg_rst": [97, J * LQ], "dbg_cat0": [P, LQ], "dbg_cat1": [HD, LQ],
        }
        r_keys = {"dbg_qt", "dbg_kt0", "dbg_va0", "dbg_exp", "dbg_cat0", "dbg_cat1"}
        for k, shp in dbg_shapes.items():
            aps[k] = nc.dram_tensor(k, shp, F32R if k in r_keys else F32,
                                    kind="ExternalOutput").ap()
    with tile.TileContext(nc) as tc:
        _emit(tc, aps, dbg=dbg)
    nc.compile()
    _CACHE[key] = nc
    return nc


def _prep_in_maps(inputs):
    f32 = np.float32
    q = np.ascontiguousarray(np.asarray(inputs["query_states"], f32))
    k = np.ascontiguousarray(np.asarray(inputs["key_states"], f32))
    v = np.ascontiguousarray(np.asarray(inputs["value_states"], f32))
    Wq = np.asarray(inputs["Wq"], f32)
    Wk = np.asarray(inputs["Wk"], f32)
    Wv = np.asarray(inputs["Wv"], f32)
    W1 = np.asarray(inputs["W1"], f32)
    W2 = np.asarray(inputs["W2"], f32)
    bq = np.asarray(inputs["bq"], f32)
    bk = np.asarray(inputs["bk"], f32)
    bv = np.asarray(inputs["bv"], f32)
    b1 = np.asarray(inputs["b1"], f32)
    b2 = np.asarray(inputs["b2"], f32)

    wqt = np.ascontiguousarray(Wq.T)
    wkt = np.ascontiguousarray(Wk.T)
    wvt = np.ascontiguousarray(Wv.T)
    w1t = np.ascontiguousarray(W1.T)                      # [192, 64]
    W2T = W2.T                                            # [hd1, dout]
    w2bd = np.zeros((P, P), f32)
    w2bd[:HD, :HD] = W2T
    w2bd[HD:, HD:] = W2T
    w2bd = w2bd.astype(ml_dtypes.bfloat16)
    b2bd = np.tile(np.concatenate([b2, b2]), (P, 1)).astype(f32)
    bv_bc = np.tile(bv, (P, 1)).astype(f32)

    qt_all = np.ascontiguousarray(q.transpose(0, 2, 1))         # [B, D, LQ]
    kt_all = np.ascontiguousarray(k.transpose(0, 1, 3, 2))      # [J, B, D, LK]
    vt_all = np.ascontiguousarray(v.transpose(0, 1, 3, 2))

    in_maps = []
    for c in range(N_CORES):
        sl = slice(c * B_LOC, (c + 1) * B_LOC)
        in_maps.append({
            "qt_in": np.ascontiguousarray(qt_all[sl]),
            "kt_in": np.ascontiguousarray(kt_all[:, sl]),
            "vt_in": np.ascontiguousarray(vt_all[:, sl]),
            "wqt": wqt, "wkt": wkt, "wvt": wvt,
            "w1t": w1t, "w2bd": w2bd, "b2bd": b2bd, "bv_bc": bv_bc,
            "ones_cols": np.ones((P, 4, H), f32),
            "bq": bq, "bk": bk, "b1": b1,
        })
    return in_maps


def kernel(**inputs):
    nc = _build()
    in_maps = _prep_in_maps(inputs)
    res = run_bass_kernel_spmd(nc, in_maps, core_ids=list(range(N_CORES)))
    out = np.concatenate([res.results[i]["out"] for i in range(N_CORES)], axis=0)
    return out.astype(np.float32)



# revision 5
# speedup vs baseline: 2.0780x; 2.0780x over previous
"""Trainium2 Bass kernel for nn_MeshCrossAttention (mesh cross-attention + per-head MLP).

Sharding: data-parallel over batch B=16 -> 2 batches per NeuronCore, 8 cores,
no collectives.

v2 design (vs v1 baseline at ~1.33 ms):
  - bf16 operands everywhere on the PE (fp32 PSUM accumulate). Halves DMA and
    SBUF traffic; all projection weights stay RESIDENT in SBUF (loaded once).
  - Transposed projections exactly like v1 (qT/kT via lhsT=W^T chunks), V in
    natural head-interleaved layout va [LK, 4, H*(HD+1)] with a ones column.
  - Scores stay transposed (sT [LK, LQ]; lhsT = kT head slice), exp on ScalarE
    -> eT bf16 tiles.
  - Context is accumulated in NATURAL layout: ctx[LQ, j*(HD+1)] via
    lhsT = eT chunk [LK, LQ-chunk], rhs = va slice [LK, HD+1]. The ones column
    of va makes column HD the softmax denominator, which now lives PER
    PARTITION -> normalization is a plain DVE reciprocal + tensor_scalar
    multiply. No DRAM-roundtrip partition broadcast (v1's big serializer).
  - cat [LQ, 192] is transposed back with PE identity-matmuls for the per-head
    MLP (contraction over 192 needs cat^T), then MLP1 -> Gelu -> MLP2 with the
    MLP2 output written naturally into a [LQ, D] staging tile -> single DMA out.
  - Exp and Gelu are batched in 8-head phases so the ScalarE activation table
    swaps 4x per batch instead of 52x.
  - PSUM budget: big(2) + att(2) + ctx(3, paired [128, 390]) + m64(1) = 8 banks.
"""
import math
import sys

import numpy as np

if "/opt/trn_rl_repo" not in sys.path:
    sys.path.insert(0, "/opt/trn_rl_repo")

import ml_dtypes  # noqa: E402

import concourse.bass as bass  # noqa: E402
import concourse.tile as tile  # noqa: E402
from concourse import bacc, mybir  # noqa: E402
from concourse.bass_utils import run_bass_kernel_spmd  # noqa: E402

F32 = mybir.dt.float32
BF16 = mybir.dt.bfloat16

D, H, HD, J = 1024, 16, 64, 3
B, LQ, LK = 16, 512, 512
P = 128
N_CORES = 8
B_LOC = B // N_CORES  # 2
E = HD + 1            # 65: head stripe width in va (ones column at HD)
HG = 8                # heads per exp/gelu phase group


def _emit(tc, aps, dbg=False):
    nc = tc.nc
    ctx_mgr = []

    def pool(name, bufs, space="SBUF"):
        p = tc.tile_pool(name=name, bufs=bufs, space=space)
        ctx_mgr.append(p)
        return p.__enter__()

    const = pool("const", 1)
    ain = pool("ain", 10)          # streamed activation chunks [128, 512] bf16
    qt_pool = pool("qt", 16)       # qT tiles, double-buffered across batches
    kt_pool = pool("kt", 24)
    va_pool = pool("va", 3)
    expp = pool("expp", 6)
    recp = pool("recp", 4)
    catp = pool("catp", 36)        # normalized cat stash for one 8-head group
    ctp = pool("ctp", 2)
    ctp1 = pool("ctp1", 2)
    h1p = pool("h1p", 2)
    ostg = pool("ostg", 5)

    ps_big = pool("ps_big", 2, "PSUM")   # projection accumulators [128, 512]
    ps_att = pool("ps_att", 2, "PSUM")   # scores / transposes / mlp2 [128, 512]
    ps_ctx = pool("ps_ctx", 3, "PSUM")   # ctx pairs [128, 2*J*E = 390]
    ps_m64 = pool("ps_m64", 1, "PSUM")   # [64, 512]: pt1 + mlp1

    # ---------------- resident constants ----------------
    wq_sb, wk_sb, wv_sb = [], [], []
    for nm, lst in (("wqt", wq_sb), ("wkt", wk_sb), ("wvt", wv_sb)):
        for i in range(8):
            t = const.tile([P, D], BF16, tag=f"{nm}{i}", name=f"{nm}{i}")
            nc.sync.dma_start(out=t[:], in_=aps[nm][i * P:(i + 1) * P, :])
            lst.append(t)

    w1a = const.tile([P, HD], BF16, tag="w1a", name="w1a")
    nc.sync.dma_start(out=w1a[:], in_=aps["w1t"][0:P, :])
    w1b = const.tile([HD, HD], BF16, tag="w1b", name="w1b")
    nc.sync.dma_start(out=w1b[:], in_=aps["w1t"][P:J * HD, :])
    w2t = const.tile([HD, HD], BF16, tag="w2t", name="w2t")
    nc.sync.dma_start(out=w2t[:], in_=aps["w2t"][:, :])
    ident = const.tile([P, P], BF16, tag="ident", name="ident")
    nc.sync.dma_start(out=ident[:], in_=aps["ident"][:, :])

    bq_sb = const.tile([P, 8], F32, tag="bq", name="bq_sb")
    nc.sync.dma_start(out=bq_sb[:], in_=aps["bq"][:, :])
    bk_sb = const.tile([P, 8], F32, tag="bk", name="bk_sb")
    nc.sync.dma_start(out=bk_sb[:], in_=aps["bk"][:, :])
    bv_bc = const.tile([P, D], BF16, tag="bv", name="bv_bc")
    nc.sync.dma_start(out=bv_bc[:], in_=aps["bv_bc"][:, :])
    b2_bc = const.tile([P, D], F32, tag="b2", name="b2_bc")
    nc.sync.dma_start(out=b2_bc[:], in_=aps["b2_bc"][:, :])
    b1_sb = const.tile([HD, 1], F32, tag="b1", name="b1_sb")
    nc.sync.dma_start(out=b1_sb[:], in_=aps["b1"][:, :])

    def load_acts(ap_slice):
        ts = []
        for ic in range(8):
            t = ain.tile([P, 512], BF16, tag="ain", name="act")
            nc.sync.dma_start(out=t[:], in_=ap_slice[ic * P:(ic + 1) * P, :])
            ts.append(t)
        return ts

    def proj_T(w_tiles, x_tiles, bias_sb, out_pool, out_tag):
        """out[oc] [128, 512] = (W @ x^T) chunk + bias, bf16."""
        outs = []
        for oc in range(8):
            pss = ps_big.tile([P, 512], F32, tag="big", name="pss")
            for ic in range(8):
                nc.tensor.matmul(
                    out=pss[:], lhsT=w_tiles[ic][:, oc * P:(oc + 1) * P],
                    rhs=x_tiles[ic][:], start=(ic == 0), stop=(ic == 7))
            t = out_pool.tile([P, 512], BF16, tag=out_tag, name=out_tag)
            nc.vector.tensor_scalar_add(t[:], pss[:], bias_sb[:, oc:oc + 1])
            outs.append(t)
        return outs

    def proj_V(x_tiles, va):
        """va [128, 4, H*E] natural head-interleaved V + ones column."""
        nc.sync.dma_start(
            out=va.rearrange("p c (h e) -> p c h e", e=E)[:, :, :, HD],
            in_=aps["ones_cols"][:, :, :])
        for half in range(2):
            for nck in range(4):
                pss = ps_big.tile([P, 512], F32, tag="big", name="pssv")
                for ic in range(8):
                    nc.tensor.matmul(
                        out=pss[:],
                        lhsT=x_tiles[ic][:, nck * P:(nck + 1) * P],
                        rhs=wv_sb[ic][:, half * 512:(half + 1) * 512],
                        start=(ic == 0), stop=(ic == 7))
                dst = va[:, nck, :].rearrange("p (h e) -> p h e", e=E)[
                    :, half * 8:(half + 1) * 8, 0:HD]
                nc.vector.tensor_tensor(
                    out=dst,
                    in0=pss[:].rearrange("p (h e) -> p h e", e=HD),
                    in1=bv_bc[:, half * 512:(half + 1) * 512].rearrange(
                        "p (h e) -> p h e", e=HD),
                    op=mybir.AluOpType.add)

    for b in range(B_LOC):
        # ================= projections =================
        qin = load_acts(aps["qt_in"][b])
        qT = proj_T(wq_sb, qin, bq_sb, qt_pool, "qt")

        kT = []
        for j in range(J):
            kin = load_acts(aps["kt_in"][j, b])
            kT.append(proj_T(wk_sb, kin, bk_sb, kt_pool, "kt"))

        va_list = []
        for j in range(J):
            vin = load_acts(aps["vt_in"][j, b])
            va = va_pool.tile([P, 4, H * E], BF16, tag="va", name="va")
            proj_V(vin, va)
            va_list.append(va)

        if dbg and b == 0:
            for oc in range(8):
                nc.sync.dma_start(out=aps["dbg_qt"][oc], in_=qT[oc][:])
                nc.sync.dma_start(out=aps["dbg_kt0"][oc], in_=kT[0][oc][:])
            nc.sync.dma_start(out=aps["dbg_va0"][:, :, :], in_=va_list[0][:])

        ost = [ostg.tile([P, D], F32, tag="ostg", name=f"ost{i}")
               for i in range(4)]

        # ================= attention + MLP, 8-head phases =================
        for hg in range(H // HG):
            cat_all = {}
            for h in range(hg * HG, (hg + 1) * HG):
                ti, r0 = h // 2, (h % 2) * HD
                pscs = [ps_ctx.tile([P, 2 * J * E], F32, tag="ctx",
                                    name=f"psc{i}") for i in range(2)]
                for j in range(J):
                    for ci in range(4):
                        pss = ps_att.tile([P, LQ], F32, tag="att", name="ps_s")
                        nc.tensor.matmul(
                            out=pss[:],
                            lhsT=kT[j][ti][r0:r0 + HD, ci * P:(ci + 1) * P],
                            rhs=qT[ti][r0:r0 + HD, :], start=True, stop=True)
                        et = expp.tile([P, LQ], BF16, tag="expp", name="et")
                        nc.scalar.activation(
                            out=et[:], in_=pss[:],
                            func=mybir.ActivationFunctionType.Exp,
                            scale=1.0 / math.sqrt(HD))
                        if dbg and b == 0 and h == 0 and j == 0:
                            nc.sync.dma_start(out=aps["dbg_exp"][ci], in_=et[:])
                        for lqc in range(4):
                            # One start/stop per PSUM bank: start lazily
                            # zeroes the whole 2KB bank, so only the first
                            # matmul touching each pair-tile starts the group.
                            nc.tensor.matmul(
                                out=pscs[lqc // 2][
                                    :, (lqc % 2) * J * E + j * E:
                                    (lqc % 2) * J * E + (j + 1) * E],
                                lhsT=et[:, lqc * P:(lqc + 1) * P],
                                rhs=va_list[j][:, ci, h * E:(h + 1) * E],
                                start=(j == 0 and ci == 0 and lqc % 2 == 0),
                                stop=(j == J - 1 and ci == 3 and lqc % 2 == 1))
                # normalize: denominator is column HD of each head stripe
                for pi in range(2):
                    rec = recp.tile([P, 2 * J], F32, tag="rec", name="rec")
                    nc.vector.reciprocal(
                        rec[:],
                        pscs[pi].rearrange("p (x e) -> p x e", e=E)[:, :, HD])
                    for half in range(2):
                        lqc = pi * 2 + half
                        cat = catp.tile([P, J * HD], BF16, tag="cat",
                                        name="cat")
                        for j in range(J):
                            nc.vector.tensor_scalar_mul(
                                cat[:, j * HD:(j + 1) * HD],
                                pscs[pi][:, half * J * E + j * E:
                                         half * J * E + j * E + HD],
                                rec[:, half * J + j:half * J + j + 1])
                        cat_all[(h, lqc)] = cat
                    if dbg and b == 0 and h == 0:
                        nc.sync.dma_start(out=aps["dbg_rec"][pi], in_=rec[:])
            if dbg and b == 0 and hg == 0:
                for lqc in range(4):
                    nc.sync.dma_start(out=aps["dbg_cat"][lqc],
                                      in_=cat_all[(0, lqc)][:])

            # ---- MLP for the 8-head group ----
            for h in range(hg * HG, (hg + 1) * HG):
                pt0 = ps_att.tile([P, LQ], F32, tag="att", name="pt0")
                pt1 = ps_m64.tile([HD, LQ], F32, tag="m64", name="pt1")
                for lqc in range(4):
                    nc.tensor.matmul(
                        out=pt0[:, lqc * P:(lqc + 1) * P],
                        lhsT=cat_all[(h, lqc)][:, 0:P], rhs=ident[:],
                        start=(lqc == 0), stop=(lqc == 3))
                    nc.tensor.matmul(
                        out=pt1[:, lqc * P:(lqc + 1) * P],
                        lhsT=cat_all[(h, lqc)][:, P:J * HD], rhs=ident[:],
                        start=(lqc == 0), stop=(lqc == 3))
                catT0 = ctp.tile([P, LQ], BF16, tag="ct0", name="catT0")
                catT1 = ctp1.tile([HD, LQ], BF16, tag="ct1", name="catT1")
                nc.vector.tensor_copy(out=catT0[:], in_=pt0[:])
                nc.vector.tensor_copy(out=catT1[:], in_=pt1[:])
                ph1 = ps_m64.tile([HD, LQ], F32, tag="m64", name="ph1")
                nc.tensor.matmul(out=ph1[:], lhsT=w1a[:], rhs=catT0[:],
                                 start=True, stop=False)
                nc.tensor.matmul(out=ph1[:], lhsT=w1b[:], rhs=catT1[:],
                                 start=False, stop=True)
                h1 = h1p.tile([HD, LQ], BF16, tag="h1", name="h1")
                nc.scalar.activation(
                    out=h1[:], in_=ph1[:],
                    func=mybir.ActivationFunctionType.Gelu, bias=b1_sb[:])
                if dbg and b == 0 and h == 0:
                    nc.sync.dma_start(out=aps["dbg_catT0"][:, :], in_=catT0[:])
                    nc.sync.dma_start(out=aps["dbg_h1"][:, :], in_=h1[:])
                for lqc in range(4):
                    ps2 = ps_att.tile([P, LQ], F32, tag="att", name="ps2")
                    nc.tensor.matmul(
                        out=ps2[:, 0:HD], lhsT=h1[:, lqc * P:(lqc + 1) * P],
                        rhs=w2t[:], start=True, stop=True)
                    nc.vector.tensor_add(
                        ost[lqc][:, h * HD:(h + 1) * HD], ps2[:, 0:HD],
                        b2_bc[:, h * HD:(h + 1) * HD])

        for lqc in range(4):
            nc.sync.dma_start(out=aps["out"][b, lqc * P:(lqc + 1) * P, :],
                              in_=ost[lqc][:])

    for p in reversed(ctx_mgr):
        p.__exit__(None, None, None)


_CACHE = {}


def _build(dbg=False):
    key = ("nc", dbg)
    if key in _CACHE:
        return _CACHE[key]
    nc = bacc.Bacc("TRN2", target_bir_lowering=False, debug=False)
    shapes = {
        "qt_in": ([B_LOC, D, LQ], BF16),
        "kt_in": ([J, B_LOC, D, LK], BF16),
        "vt_in": ([J, B_LOC, D, LK], BF16),
        "wqt": ([D, D], BF16),
        "wkt": ([D, D], BF16),
        "wvt": ([D, D], BF16),
        "w1t": ([J * HD, HD], BF16),
        "w2t": ([HD, HD], BF16),
        "ident": ([P, P], BF16),
        "ones_cols": ([P, 4, H], BF16),
        "bq": ([P, 8], F32),
        "bk": ([P, 8], F32),
        "bv_bc": ([P, D], BF16),
        "b2_bc": ([P, D], F32),
        "b1": ([HD, 1], F32),
    }
    aps = {k: nc.dram_tensor(k, s, dt, kind="ExternalInput").ap()
           for k, (s, dt) in shapes.items()}
    aps["out"] = nc.dram_tensor("out", [B_LOC, LQ, D], F32,
                                kind="ExternalOutput").ap()
    if dbg:
        dbg_shapes = {
            "dbg_qt": ([8, P, 512], BF16), "dbg_kt0": ([8, P, 512], BF16),
            "dbg_va0": ([P, 4, H * E], BF16), "dbg_exp": ([4, P, LQ], BF16),
            "dbg_rec": ([2, P, 2 * J], F32), "dbg_cat": ([4, P, J * HD], BF16),
            "dbg_catT0": ([P, LQ], BF16), "dbg_h1": ([HD, LQ], BF16),
        }
        for k, (shp, dt) in dbg_shapes.items():
            aps[k] = nc.dram_tensor(k, shp, dt, kind="ExternalOutput").ap()
    with tile.TileContext(nc) as tc:
        _emit(tc, aps, dbg=dbg)
    nc.compile()
    _CACHE[key] = nc
    return nc


def _prep_in_maps(inputs):
    f32 = np.float32
    bf16 = ml_dtypes.bfloat16
    q = np.ascontiguousarray(np.asarray(inputs["query_states"], f32))
    k = np.ascontiguousarray(np.asarray(inputs["key_states"], f32))
    v = np.ascontiguousarray(np.asarray(inputs["value_states"], f32))
    Wq = np.asarray(inputs["Wq"], f32)
    Wk = np.asarray(inputs["Wk"], f32)
    Wv = np.asarray(inputs["Wv"], f32)
    W1 = np.asarray(inputs["W1"], f32)
    W2 = np.asarray(inputs["W2"], f32)
    bq = np.asarray(inputs["bq"], f32)
    bk = np.asarray(inputs["bk"], f32)
    bv = np.asarray(inputs["bv"], f32)
    b1 = np.asarray(inputs["b1"], f32)
    b2 = np.asarray(inputs["b2"], f32)

    wqt = np.ascontiguousarray(Wq.T).astype(bf16)
    wkt = np.ascontiguousarray(Wk.T).astype(bf16)
    wvt = np.ascontiguousarray(Wv.T).astype(bf16)
    w1t = np.ascontiguousarray(W1.T).astype(bf16)          # [192, 64]
    w2t = np.ascontiguousarray(W2.T).astype(bf16)          # [64, 64]
    ident = np.eye(P, dtype=f32).astype(bf16)
    bq_sb = np.ascontiguousarray(bq.reshape(8, P).T).astype(f32)
    bk_sb = np.ascontiguousarray(bk.reshape(8, P).T).astype(f32)
    bv_bc = np.tile(bv, (P, 1)).astype(bf16)
    b2_bc = np.tile(b2, (P, H)).astype(f32)
    b1_col = b1.reshape(HD, 1).astype(f32)
    ones_cols = np.ones((P, 4, H), f32).astype(bf16)

    qt_all = np.ascontiguousarray(q.transpose(0, 2, 1)).astype(bf16)
    kt_all = np.ascontiguousarray(k.transpose(0, 1, 3, 2)).astype(bf16)
    vt_all = np.ascontiguousarray(v.transpose(0, 1, 3, 2)).astype(bf16)

    in_maps = []
    for c in range(N_CORES):
        sl = slice(c * B_LOC, (c + 1) * B_LOC)
        in_maps.append({
            "qt_in": np.ascontiguousarray(qt_all[sl]),
            "kt_in": np.ascontiguousarray(kt_all[:, sl]),
            "vt_in": np.ascontiguousarray(vt_all[:, sl]),
            "wqt": wqt, "wkt": wkt, "wvt": wvt,
            "w1t": w1t, "w2t": w2t, "ident": ident,
            "ones_cols": ones_cols,
            "bq": bq_sb, "bk": bk_sb, "bv_bc": bv_bc,
            "b2_bc": b2_bc, "b1": b1_col,
        })
    return in_maps


def kernel(**inputs):
    nc = _build()
    in_maps = _prep_in_maps(inputs)
    res = run_bass_kernel_spmd(nc, in_maps, core_ids=list(range(N_CORES)))
    out = np.concatenate([res.results[i]["out"] for i in range(N_CORES)], axis=0)
    return out.astype(np.float32)


# revision 9
# speedup vs baseline: 2.0801x; 1.0010x over previous
"""Trainium2 Bass kernel for nn_MeshCrossAttention (mesh cross-attention + per-head MLP).

Sharding: data-parallel over batch B=16 -> 2 batches per NeuronCore, 8 cores,
no collectives.

v2 design (vs v1 baseline at ~1.33 ms):
  - bf16 operands everywhere on the PE (fp32 PSUM accumulate). Halves DMA and
    SBUF traffic; all projection weights stay RESIDENT in SBUF (loaded once).
  - Transposed projections exactly like v1 (qT/kT via lhsT=W^T chunks), V in
    natural head-interleaved layout va [LK, 4, H*(HD+1)] with a ones column.
  - Scores stay transposed (sT [LK, LQ]; lhsT = kT head slice), exp on ScalarE
    -> eT bf16 tiles.
  - Context is accumulated in NATURAL layout: ctx[LQ, j*(HD+1)] via
    lhsT = eT chunk [LK, LQ-chunk], rhs = va slice [LK, HD+1]. The ones column
    of va makes column HD the softmax denominator, which now lives PER
    PARTITION -> normalization is a plain DVE reciprocal + tensor_scalar
    multiply. No DRAM-roundtrip partition broadcast (v1's big serializer).
  - cat [LQ, 192] is transposed back with PE identity-matmuls for the per-head
    MLP (contraction over 192 needs cat^T), then MLP1 -> Gelu -> MLP2 with the
    MLP2 output written naturally into a [LQ, D] staging tile -> single DMA out.
  - Exp and Gelu are batched in 8-head phases so the ScalarE activation table
    swaps 4x per batch instead of 52x.
  - PSUM budget: big(2) + att(2) + ctx(3, paired [128, 390]) + m64(1) = 8 banks.
"""
import math
import sys

import numpy as np

if "/opt/trn_rl_repo" not in sys.path:
    sys.path.insert(0, "/opt/trn_rl_repo")

import ml_dtypes  # noqa: E402

import concourse.bass as bass  # noqa: E402
import concourse.tile as tile  # noqa: E402
from concourse import bacc, mybir  # noqa: E402
from concourse.bass_utils import run_bass_kernel_spmd  # noqa: E402

F32 = mybir.dt.float32
BF16 = mybir.dt.bfloat16

D, H, HD, J = 1024, 16, 64, 3
B, LQ, LK = 16, 512, 512
P = 128
N_CORES = 8
B_LOC = B // N_CORES  # 2
E = HD + 1            # 65: head stripe width in va (ones column at HD)
HG = 8                # heads per exp/gelu phase group


def _emit(tc, aps, dbg=False):
    nc = tc.nc
    ctx_mgr = []

    def pool(name, bufs, space="SBUF"):
        p = tc.tile_pool(name=name, bufs=bufs, space=space)
        ctx_mgr.append(p)
        return p.__enter__()

    const = pool("const", 1)
    ain = pool("ain", 10)          # streamed activation chunks [128, 512] bf16
    qt_pool = pool("qt", 16)       # qT tiles, double-buffered across batches
    kt_pool = pool("kt", 24)
    va_pool = pool("va", 3)
    expp = pool("expp", 8)
    recp = pool("recp", 4)
    catp = pool("catp", 10)        # normalized cat, consumed by next head's fillers
    ctp = pool("ctp", 10)          # catT halves, alive until the hg mini-MLP phase
    ctp1 = pool("ctp1", 10)
    h1p = pool("h1p", 2)
    ostg = pool("ostg", 5)

    ps_big = pool("ps_big", 2, "PSUM")   # proj accumulators + cat transposes
    ps_att = pool("ps_att", 2, "PSUM")   # scores / mlp2 [128, 512]
    ps_ctx = pool("ps_ctx", 2, "PSUM")   # ctx pairs [128, 2*J*E = 390]
    ps_m64 = pool("ps_m64", 2, "PSUM")   # mlp1 accumulators

    # ---------------- resident constants ----------------
    wq_sb, wk_sb, wv_sb = [], [], []
    for nm, lst in (("wqt", wq_sb), ("wkt", wk_sb), ("wvt", wv_sb)):
        for i in range(8):
            t = const.tile([P, D], BF16, tag=f"{nm}{i}", name=f"{nm}{i}")
            nc.sync.dma_start(out=t[:], in_=aps[nm][i * P:(i + 1) * P, :])
            lst.append(t)

    w1a = const.tile([P, HD], BF16, tag="w1a", name="w1a")
    nc.sync.dma_start(out=w1a[:], in_=aps["w1t"][0:P, :])
    w1b = const.tile([HD, HD], BF16, tag="w1b", name="w1b")
    nc.sync.dma_start(out=w1b[:], in_=aps["w1t"][P:J * HD, :])
    w2t = const.tile([HD, HD], BF16, tag="w2t", name="w2t")
    nc.sync.dma_start(out=w2t[:], in_=aps["w2t"][:, :])
    ident = const.tile([P, P], BF16, tag="ident", name="ident")
    nc.sync.dma_start(out=ident[:], in_=aps["ident"][:, :])

    bq_sb = const.tile([P, 8], F32, tag="bq", name="bq_sb")
    nc.sync.dma_start(out=bq_sb[:], in_=aps["bq"][:, :])
    bk_sb = const.tile([P, 8], F32, tag="bk", name="bk_sb")
    nc.sync.dma_start(out=bk_sb[:], in_=aps["bk"][:, :])
    bv_bc = const.tile([P, D], BF16, tag="bv", name="bv_bc")
    nc.sync.dma_start(out=bv_bc[:], in_=aps["bv_bc"][:, :])
    b2_bc = const.tile([P, D], F32, tag="b2", name="b2_bc")
    nc.sync.dma_start(out=b2_bc[:], in_=aps["b2_bc"][:, :])
    b1_sb = const.tile([HD, 1], F32, tag="b1", name="b1_sb")
    nc.sync.dma_start(out=b1_sb[:], in_=aps["b1"][:, :])

    def load_acts(ap_slice):
        ts = []
        for ic in range(8):
            t = ain.tile([P, 512], BF16, tag="ain", name="act")
            nc.sync.dma_start(out=t[:], in_=ap_slice[ic * P:(ic + 1) * P, :])
            ts.append(t)
        return ts

    def proj_T(w_tiles, x_tiles, bias_sb, out_pool, out_tag):
        """out[oc] [128, 512] = (W @ x^T) chunk + bias, bf16."""
        outs = []
        for oc in range(8):
            pss = ps_big.tile([P, 512], F32, tag="big", name="pss")
            for ic in range(8):
                nc.tensor.matmul(
                    out=pss[:], lhsT=w_tiles[ic][:, oc * P:(oc + 1) * P],
                    rhs=x_tiles[ic][:], start=(ic == 0), stop=(ic == 7))
            t = out_pool.tile([P, 512], BF16, tag=out_tag, name=out_tag)
            nc.vector.tensor_scalar_add(t[:], pss[:], bias_sb[:, oc:oc + 1])
            outs.append(t)
        return outs

    def proj_V(x_tiles, va):
        """va [128, 4, H*E] natural head-interleaved V + ones column."""
        nc.sync.dma_start(
            out=va.rearrange("p c (h e) -> p c h e", e=E)[:, :, :, HD],
            in_=aps["ones_cols"][:, :, :])
        for half in range(2):
            for nck in range(4):
                pss = ps_big.tile([P, 512], F32, tag="big", name="pssv")
                for ic in range(8):
                    nc.tensor.matmul(
                        out=pss[:],
                        lhsT=x_tiles[ic][:, nck * P:(nck + 1) * P],
                        rhs=wv_sb[ic][:, half * 512:(half + 1) * 512],
                        start=(ic == 0), stop=(ic == 7))
                dst = va[:, nck, :].rearrange("p (h e) -> p h e", e=E)[
                    :, half * 8:(half + 1) * 8, 0:HD]
                nc.vector.tensor_tensor(
                    out=dst,
                    in0=pss[:].rearrange("p (h e) -> p h e", e=HD),
                    in1=bv_bc[:, half * 512:(half + 1) * 512].rearrange(
                        "p (h e) -> p h e", e=HD),
                    op=mybir.AluOpType.add)

    for b in range(B_LOC):
        # ================= projections =================
        qin = load_acts(aps["qt_in"][b])
        qT = proj_T(wq_sb, qin, bq_sb, qt_pool, "qt")

        kT = []
        for j in range(J):
            kin = load_acts(aps["kt_in"][j, b])
            kT.append(proj_T(wk_sb, kin, bk_sb, kt_pool, "kt"))

        va_list = []
        for j in range(J):
            vin = load_acts(aps["vt_in"][j, b])
            va = va_pool.tile([P, 4, H * E], BF16, tag="va", name="va")
            proj_V(vin, va)
            va_list.append(va)

        if dbg and b == 0:
            for oc in range(8):
                nc.sync.dma_start(out=aps["dbg_qt"][oc], in_=qT[oc][:])
                nc.sync.dma_start(out=aps["dbg_kt0"][oc], in_=kT[0][oc][:])
            nc.sync.dma_start(out=aps["dbg_va0"][:, :, :], in_=va_list[0][:])

        ost = [ostg.tile([P, D], F32, tag="ostg", name=f"ost{i}")
               for i in range(4)]

        # ================= attention + MLP, 8-head phases =================
        # Per head: 12 (score -> exp) steps; ctx matmuls consume the PREVIOUS
        # step's exp output so the PE never waits on ScalarE. The previous
        # head's cat transposes are interleaved one-per-step as PE filler
        # during exp latency. MLP1/Gelu/MLP2 run as a compact per-group phase
        # (2 activation-table swaps per group).
        def emit_transposes(h_prev, cat_tiles):
            """Returns a list of thunks: 8 PE transposes + 2 copies."""
            pt0 = ps_big.tile([P, LQ], F32, tag="big", name="pt0")
            pt1 = ps_big.tile([P, LQ], F32, tag="big", name="pt1")
            thunks = []
            for lqc in range(4):
                thunks.append(lambda lqc=lqc: nc.tensor.matmul(
                    out=pt0[:, lqc * P:(lqc + 1) * P],
                    lhsT=cat_tiles[lqc][:, 0:P], rhs=ident[:],
                    start=(lqc == 0), stop=(lqc == 3)))
                thunks.append(lambda lqc=lqc: nc.tensor.matmul(
                    out=pt1[0:HD, lqc * P:(lqc + 1) * P],
                    lhsT=cat_tiles[lqc][:, P:J * HD], rhs=ident[:],
                    start=(lqc == 0), stop=(lqc == 3)))
            catT0 = ctp.tile([P, LQ], BF16, tag="ct0", name="catT0")
            catT1 = ctp1.tile([HD, LQ], BF16, tag="ct1", name="catT1")
            thunks.append(lambda: nc.vector.tensor_copy(out=catT0[:],
                                                        in_=pt0[:]))
            thunks.append(lambda: nc.vector.tensor_copy(out=catT1[:],
                                                        in_=pt1[0:HD, :]))
            catT_all[h_prev] = (catT0, catT1)
            return thunks

        for hg in range(H // HG):
            cat_all = {}
            catT_all = {}
            fillers = []
            for h in range(hg * HG, (hg + 1) * HG):
                ti, r0 = h // 2, (h % 2) * HD
                pscs = [ps_ctx.tile([P, 2 * J * E], F32, tag="ctx",
                                    name=f"psc{i}") for i in range(2)]

                def emit_ctx(j, ci, et):
                    for lqc in range(4):
                        # One start/stop per PSUM bank: start lazily zeroes
                        # the whole 2KB bank, so only the first matmul
                        # touching each pair-tile starts the group.
                        nc.tensor.matmul(
                            out=pscs[lqc // 2][
                                :, (lqc % 2) * J * E + j * E:
                                (lqc % 2) * J * E + (j + 1) * E],
                            lhsT=et[:, lqc * P:(lqc + 1) * P],
                            rhs=va_list[j][:, ci, h * E:(h + 1) * E],
                            start=(j == 0 and ci == 0 and lqc % 2 == 0),
                            stop=(j == J - 1 and ci == 3 and lqc % 2 == 1))

                prev = None
                for j in range(J):
                    for ci in range(4):
                        pss = ps_att.tile([P, LQ], F32, tag="att", name="ps_s")
                        nc.tensor.matmul(
                            out=pss[:],
                            lhsT=kT[j][ti][r0:r0 + HD, ci * P:(ci + 1) * P],
                            rhs=qT[ti][r0:r0 + HD, :], start=True, stop=True)
                        et = expp.tile([P, LQ], BF16, tag="expp", name="et")
                        nc.scalar.activation(
                            out=et[:], in_=pss[:],
                            func=mybir.ActivationFunctionType.Exp,
                            scale=1.0 / math.sqrt(HD))
                        if dbg and b == 0 and h == 0 and j == 0:
                            nc.sync.dma_start(out=aps["dbg_exp"][ci], in_=et[:])
                        if fillers:
                            fillers.pop(0)()
                        if prev is not None:
                            emit_ctx(*prev)
                        prev = (j, ci, et)
                emit_ctx(*prev)
                while fillers:
                    fillers.pop(0)()

                # normalize: denominator is column HD of each head stripe
                cat_tiles = []
                for pi in range(2):
                    rec = recp.tile([P, 2 * J], F32, tag="rec", name="rec")
                    nc.vector.reciprocal(
                        rec[:],
                        pscs[pi].rearrange("p (x e) -> p x e", e=E)[:, :, HD])
                    for half in range(2):
                        cat = catp.tile([P, J * HD], BF16, tag="cat",
                                        name="cat")
                        for j in range(J):
                            nc.vector.tensor_scalar_mul(
                                cat[:, j * HD:(j + 1) * HD],
                                pscs[pi][:, half * J * E + j * E:
                                         half * J * E + j * E + HD],
                                rec[:, half * J + j:half * J + j + 1])
                        cat_tiles.append(cat)
                    if dbg and b == 0 and h == 0:
                        nc.sync.dma_start(out=aps["dbg_rec"][pi], in_=rec[:])
                cat_all[h] = cat_tiles
                fillers = emit_transposes(h, cat_tiles)
            while fillers:
                fillers.pop(0)()
            if dbg and b == 0 and hg == 0:
                for lqc in range(4):
                    nc.sync.dma_start(out=aps["dbg_cat"][lqc],
                                      in_=cat_all[0][lqc][:])

            # ---- MLP1 / Gelu / MLP2 for the 8-head group ----
            for h in range(hg * HG, (hg + 1) * HG):
                catT0, catT1 = catT_all[h]
                ph1 = ps_m64.tile([P, LQ], F32, tag="m64", name="ph1")
                nc.tensor.matmul(out=ph1[0:HD, :], lhsT=w1a[:], rhs=catT0[:],
                                 start=True, stop=False)
                nc.tensor.matmul(out=ph1[0:HD, :], lhsT=w1b[:], rhs=catT1[:],
                                 start=False, stop=True)
                h1 = h1p.tile([HD, LQ], BF16, tag="h1", name="h1")
                nc.scalar.activation(
                    out=h1[:], in_=ph1[0:HD, :],
                    func=mybir.ActivationFunctionType.Gelu, bias=b1_sb[:])
                if dbg and b == 0 and h == 0:
                    nc.sync.dma_start(out=aps["dbg_catT0"][:, :], in_=catT0[:])
                    nc.sync.dma_start(out=aps["dbg_h1"][:, :], in_=h1[:])
                for lqc in range(4):
                    ps2 = ps_att.tile([P, LQ], F32, tag="att", name="ps2")
                    nc.tensor.matmul(
                        out=ps2[:, 0:HD], lhsT=h1[:, lqc * P:(lqc + 1) * P],
                        rhs=w2t[:], start=True, stop=True)
                    nc.vector.tensor_add(
                        ost[lqc][:, h * HD:(h + 1) * HD], ps2[:, 0:HD],
                        b2_bc[:, h * HD:(h + 1) * HD])

        for lqc in range(4):
            nc.sync.dma_start(out=aps["out"][b, lqc * P:(lqc + 1) * P, :],
                              in_=ost[lqc][:])

    for p in reversed(ctx_mgr):
        p.__exit__(None, None, None)


_CACHE = {}


def _build(dbg=False):
    key = ("nc", dbg)
    if key in _CACHE:
        return _CACHE[key]
    nc = bacc.Bacc("TRN2", target_bir_lowering=False, debug=False)
    shapes = {
        "qt_in": ([B_LOC, D, LQ], BF16),
        "kt_in": ([J, B_LOC, D, LK], BF16),
        "vt_in": ([J, B_LOC, D, LK], BF16),
        "wqt": ([D, D], BF16),
        "wkt": ([D, D], BF16),
        "wvt": ([D, D], BF16),
        "w1t": ([J * HD, HD], BF16),
        "w2t": ([HD, HD], BF16),
        "ident": ([P, P], BF16),
        "ones_cols": ([P, 4, H], BF16),
        "bq": ([P, 8], F32),
        "bk": ([P, 8], F32),
        "bv_bc": ([P, D], BF16),
        "b2_bc": ([P, D], F32),
        "b1": ([HD, 1], F32),
    }
    aps = {k: nc.dram_tensor(k, s, dt, kind="ExternalInput").ap()
           for k, (s, dt) in shapes.items()}
    aps["out"] = nc.dram_tensor("out", [B_LOC, LQ, D], F32,
                                kind="ExternalOutput").ap()
    if dbg:
        dbg_shapes = {
            "dbg_qt": ([8, P, 512], BF16), "dbg_kt0": ([8, P, 512], BF16),
            "dbg_va0": ([P, 4, H * E], BF16), "dbg_exp": ([4, P, LQ], BF16),
            "dbg_rec": ([2, P, 2 * J], F32), "dbg_cat": ([4, P, J * HD], BF16),
            "dbg_catT0": ([P, LQ], BF16), "dbg_h1": ([HD, LQ], BF16),
        }
        for k, (shp, dt) in dbg_shapes.items():
            aps[k] = nc.dram_tensor(k, shp, dt, kind="ExternalOutput").ap()
    with tile.TileContext(nc) as tc:
        _emit(tc, aps, dbg=dbg)
    nc.compile()
    _CACHE[key] = nc
    return nc


def _prep_in_maps(inputs):
    f32 = np.float32
    bf16 = ml_dtypes.bfloat16
    q = np.ascontiguousarray(np.asarray(inputs["query_states"], f32))
    k = np.ascontiguousarray(np.asarray(inputs["key_states"], f32))
    v = np.ascontiguousarray(np.asarray(inputs["value_states"], f32))
    Wq = np.asarray(inputs["Wq"], f32)
    Wk = np.asarray(inputs["Wk"], f32)
    Wv = np.asarray(inputs["Wv"], f32)
    W1 = np.asarray(inputs["W1"], f32)
    W2 = np.asarray(inputs["W2"], f32)
    bq = np.asarray(inputs["bq"], f32)
    bk = np.asarray(inputs["bk"], f32)
    bv = np.asarray(inputs["bv"], f32)
    b1 = np.asarray(inputs["b1"], f32)
    b2 = np.asarray(inputs["b2"], f32)

    wqt = np.ascontiguousarray(Wq.T).astype(bf16)
    wkt = np.ascontiguousarray(Wk.T).astype(bf16)
    wvt = np.ascontiguousarray(Wv.T).astype(bf16)
    w1t = np.ascontiguousarray(W1.T).astype(bf16)          # [192, 64]
    w2t = np.ascontiguousarray(W2.T).astype(bf16)          # [64, 64]
    ident = np.eye(P, dtype=f32).astype(bf16)
    bq_sb = np.ascontiguousarray(bq.reshape(8, P).T).astype(f32)
    bk_sb = np.ascontiguousarray(bk.reshape(8, P).T).astype(f32)
    bv_bc = np.tile(bv, (P, 1)).astype(bf16)
    b2_bc = np.tile(b2, (P, H)).astype(f32)
    b1_col = b1.reshape(HD, 1).astype(f32)
    ones_cols = np.ones((P, 4, H), f32).astype(bf16)

    qt_all = np.ascontiguousarray(q.transpose(0, 2, 1)).astype(bf16)
    kt_all = np.ascontiguousarray(k.transpose(0, 1, 3, 2)).astype(bf16)
    vt_all = np.ascontiguousarray(v.transpose(0, 1, 3, 2)).astype(bf16)

    in_maps = []
    for c in range(N_CORES):
        sl = slice(c * B_LOC, (c + 1) * B_LOC)
        in_maps.append({
            "qt_in": np.ascontiguousarray(qt_all[sl]),
            "kt_in": np.ascontiguousarray(kt_all[:, sl]),
            "vt_in": np.ascontiguousarray(vt_all[:, sl]),
            "wqt": wqt, "wkt": wkt, "wvt": wvt,
            "w1t": w1t, "w2t": w2t, "ident": ident,
            "ones_cols": ones_cols,
            "bq": bq_sb, "bk": bk_sb, "bv_bc": bv_bc,
            "b2_bc": b2_bc, "b1": b1_col,
        })
    return in_maps


def kernel(**inputs):
    nc = _build()
    in_maps = _prep_in_maps(inputs)
    res = run_bass_kernel_spmd(nc, in_maps, core_ids=list(range(N_CORES)))
    out = np.concatenate([res.results[i]["out"] for i in range(N_CORES)], axis=0)
    return out.astype(np.float32)


# revision 13
# speedup vs baseline: 2.1651x; 1.0409x over previous
"""Trainium2 Bass kernel for nn_MeshCrossAttention (mesh cross-attention + per-head MLP).

Sharding: data-parallel over batch B=16 -> 2 batches per NeuronCore, 8 cores,
no collectives.

v2 design (vs v1 baseline at ~1.33 ms):
  - bf16 operands everywhere on the PE (fp32 PSUM accumulate). Halves DMA and
    SBUF traffic; all projection weights stay RESIDENT in SBUF (loaded once).
  - Transposed projections exactly like v1 (qT/kT via lhsT=W^T chunks), V in
    natural head-interleaved layout va [LK, 4, H*(HD+1)] with a ones column.
  - Scores stay transposed (sT [LK, LQ]; lhsT = kT head slice), exp on ScalarE
    -> eT bf16 tiles.
  - Context is accumulated in NATURAL layout: ctx[LQ, j*(HD+1)] via
    lhsT = eT chunk [LK, LQ-chunk], rhs = va slice [LK, HD+1]. The ones column
    of va makes column HD the softmax denominator, which now lives PER
    PARTITION -> normalization is a plain DVE reciprocal + tensor_scalar
    multiply. No DRAM-roundtrip partition broadcast (v1's big serializer).
  - cat [LQ, 192] is transposed back with PE identity-matmuls for the per-head
    MLP (contraction over 192 needs cat^T), then MLP1 -> Gelu -> MLP2 with the
    MLP2 output written naturally into a [LQ, D] staging tile -> single DMA out.
  - Exp and Gelu are batched in 8-head phases so the ScalarE activation table
    swaps 4x per batch instead of 52x.
  - PSUM budget: big(2) + att(2) + ctx(3, paired [128, 390]) + m64(1) = 8 banks.
"""
import math
import sys

import numpy as np

if "/opt/trn_rl_repo" not in sys.path:
    sys.path.insert(0, "/opt/trn_rl_repo")

import ml_dtypes  # noqa: E402

import concourse.bass as bass  # noqa: E402
import concourse.tile as tile  # noqa: E402
from concourse import bacc, mybir  # noqa: E402
from concourse.bass_utils import run_bass_kernel_spmd  # noqa: E402

F32 = mybir.dt.float32
BF16 = mybir.dt.bfloat16

D, H, HD, J = 1024, 16, 64, 3
B, LQ, LK = 16, 512, 512
P = 128
N_CORES = 8
B_LOC = B // N_CORES  # 2
E = HD + 1            # 65: head stripe width in va (ones column at HD)
HG = 8                # heads per exp/gelu phase group


def _emit(tc, aps, dbg=False):
    nc = tc.nc
    ctx_mgr = []

    def pool(name, bufs, space="SBUF"):
        p = tc.tile_pool(name=name, bufs=bufs, space=space)
        ctx_mgr.append(p)
        return p.__enter__()

    const = pool("const", 1)
    ain = pool("ain", 10)          # streamed activation chunks [128, 512] bf16
    qt_pool = pool("qt", 16)       # qT tiles, double-buffered across batches
    kt_pool = pool("kt", 24)
    va_pool = pool("va", 3)
    expp = pool("expp", 8)
    recp = pool("recp", 4)
    catp = pool("catp", 10)        # normalized cat, consumed by next head's fillers
    ctp = pool("ctp", 10)          # catT halves, alive until the hg mini-MLP phase
    ctp1 = pool("ctp1", 10)
    h1p = pool("h1p", 4)
    ostg = pool("ostg", 5)

    ps_big = pool("ps_big", 2, "PSUM")   # proj accum + cat transposes + mlp1
    ps_att = pool("ps_att", 3, "PSUM")   # scores / mlp2 [128, 512]
    ps_ctx = pool("ps_ctx", 3, "PSUM")   # ctx pairs [128, 2*J*E = 390]

    # ---------------- resident constants ----------------
    wq_sb, wk_sb, wv_sb = [], [], []
    for nm, lst in (("wqt", wq_sb), ("wkt", wk_sb), ("wvt", wv_sb)):
        for i in range(8):
            t = const.tile([P, D], BF16, tag=f"{nm}{i}", name=f"{nm}{i}")
            nc.sync.dma_start(out=t[:], in_=aps[nm][i * P:(i + 1) * P, :])
            lst.append(t)

    w1a = const.tile([P, HD], BF16, tag="w1a", name="w1a")
    nc.sync.dma_start(out=w1a[:], in_=aps["w1t"][0:P, :])
    w1b = const.tile([HD, HD], BF16, tag="w1b", name="w1b")
    nc.sync.dma_start(out=w1b[:], in_=aps["w1t"][P:J * HD, :])
    w2t = const.tile([HD, HD], BF16, tag="w2t", name="w2t")
    nc.sync.dma_start(out=w2t[:], in_=aps["w2t"][:, :])
    ident = const.tile([P, P], BF16, tag="ident", name="ident")
    nc.sync.dma_start(out=ident[:], in_=aps["ident"][:, :])

    bq_sb = const.tile([P, 8], F32, tag="bq", name="bq_sb")
    nc.sync.dma_start(out=bq_sb[:], in_=aps["bq"][:, :])
    bk_sb = const.tile([P, 8], F32, tag="bk", name="bk_sb")
    nc.sync.dma_start(out=bk_sb[:], in_=aps["bk"][:, :])
    bv_bc = const.tile([P, D], BF16, tag="bv", name="bv_bc")
    nc.sync.dma_start(out=bv_bc[:], in_=aps["bv_bc"][:, :])
    b2_bc = const.tile([P, D], F32, tag="b2", name="b2_bc")
    nc.sync.dma_start(out=b2_bc[:], in_=aps["b2_bc"][:, :])
    b1_sb = const.tile([HD, 1], F32, tag="b1", name="b1_sb")
    nc.sync.dma_start(out=b1_sb[:], in_=aps["b1"][:, :])

    def load_acts(ap_slice):
        ts = []
        for ic in range(8):
            t = ain.tile([P, 512], BF16, tag="ain", name="act")
            nc.sync.dma_start(out=t[:], in_=ap_slice[ic * P:(ic + 1) * P, :])
            ts.append(t)
        return ts

    def proj_T(w_tiles, x_tiles, bias_sb, out_pool, out_tag):
        """out[oc] [128, 512] = (W @ x^T) chunk + bias, bf16."""
        outs = []
        for oc in range(8):
            pss = ps_big.tile([P, 512], F32, tag="big", name="pss")
            for ic in range(8):
                nc.tensor.matmul(
                    out=pss[:], lhsT=w_tiles[ic][:, oc * P:(oc + 1) * P],
                    rhs=x_tiles[ic][:], start=(ic == 0), stop=(ic == 7))
            t = out_pool.tile([P, 512], BF16, tag=out_tag, name=out_tag)
            nc.vector.tensor_scalar_add(t[:], pss[:], bias_sb[:, oc:oc + 1])
            outs.append(t)
        return outs

    def proj_V(x_tiles, va):
        """va [128, 4, H*E] natural head-interleaved V + ones column."""
        nc.sync.dma_start(
            out=va.rearrange("p c (h e) -> p c h e", e=E)[:, :, :, HD],
            in_=aps["ones_cols"][:, :, :])
        for half in range(2):
            for nck in range(4):
                pss = ps_big.tile([P, 512], F32, tag="big", name="pssv")
                for ic in range(8):
                    nc.tensor.matmul(
                        out=pss[:],
                        lhsT=x_tiles[ic][:, nck * P:(nck + 1) * P],
                        rhs=wv_sb[ic][:, half * 512:(half + 1) * 512],
                        start=(ic == 0), stop=(ic == 7))
                dst = va[:, nck, :].rearrange("p (h e) -> p h e", e=E)[
                    :, half * 8:(half + 1) * 8, 0:HD]
                nc.vector.tensor_tensor(
                    out=dst,
                    in0=pss[:].rearrange("p (h e) -> p h e", e=HD),
                    in1=bv_bc[:, half * 512:(half + 1) * 512].rearrange(
                        "p (h e) -> p h e", e=HD),
                    op=mybir.AluOpType.add)

    for b in range(B_LOC):
        # ================= projections =================
        qin = load_acts(aps["qt_in"][b])
        qT = proj_T(wq_sb, qin, bq_sb, qt_pool, "qt")

        kT = []
        for j in range(J):
            kin = load_acts(aps["kt_in"][j, b])
            kT.append(proj_T(wk_sb, kin, bk_sb, kt_pool, "kt"))

        va_list = []
        for j in range(J):
            vin = load_acts(aps["vt_in"][j, b])
            va = va_pool.tile([P, 4, H * E], BF16, tag="va", name="va")
            proj_V(vin, va)
            va_list.append(va)

        if dbg and b == 0:
            for oc in range(8):
                nc.sync.dma_start(out=aps["dbg_qt"][oc], in_=qT[oc][:])
                nc.sync.dma_start(out=aps["dbg_kt0"][oc], in_=kT[0][oc][:])
            nc.sync.dma_start(out=aps["dbg_va0"][:, :, :], in_=va_list[0][:])

        ost = [ostg.tile([P, D], F32, tag="ostg", name=f"ost{i}")
               for i in range(4)]

        # ================= attention + MLP, 8-head phases =================
        # Per head: 12 (score -> exp) steps; ctx matmuls consume the PREVIOUS
        # step's exp output so the PE never waits on ScalarE. The previous
        # head's cat transposes are interleaved one-per-step as PE filler
        # during exp latency. MLP1/Gelu/MLP2 run as a compact per-group phase
        # (2 activation-table swaps per group).
        def emit_transposes(h_prev, cat_tiles):
            """Returns a list of thunks: 8 PE transposes + 2 copies."""
            pt0 = ps_big.tile([P, LQ], F32, tag="big", name="pt0")
            pt1 = ps_big.tile([P, LQ], F32, tag="big", name="pt1")
            thunks = []
            for lqc in range(4):
                thunks.append(lambda lqc=lqc: nc.tensor.matmul(
                    out=pt0[:, lqc * P:(lqc + 1) * P],
                    lhsT=cat_tiles[lqc][:, 0:P], rhs=ident[:],
                    start=(lqc == 0), stop=(lqc == 3)))
                thunks.append(lambda lqc=lqc: nc.tensor.matmul(
                    out=pt1[0:HD, lqc * P:(lqc + 1) * P],
                    lhsT=cat_tiles[lqc][:, P:J * HD], rhs=ident[:],
                    start=(lqc == 0), stop=(lqc == 3)))
            catT0 = ctp.tile([P, LQ], BF16, tag="ct0", name="catT0")
            catT1 = ctp1.tile([HD, LQ], BF16, tag="ct1", name="catT1")
            thunks.append(lambda: nc.vector.tensor_copy(out=catT0[:],
                                                        in_=pt0[:]))
            thunks.append(lambda: nc.vector.tensor_copy(out=catT1[:],
                                                        in_=pt1[0:HD, :]))
            catT_all[h_prev] = (catT0, catT1)
            return thunks

        for hg in range(H // HG):
            cat_all = {}
            catT_all = {}
            fillers = []
            for h in range(hg * HG, (hg + 1) * HG):
                ti, r0 = h // 2, (h % 2) * HD
                pscs = [ps_ctx.tile([P, 2 * J * E], F32, tag="ctx",
                                    name=f"psc{i}") for i in range(2)]

                def emit_ctx(j, ci, et):
                    for lqc in range(4):
                        # One start/stop per PSUM bank: start lazily zeroes
                        # the whole 2KB bank, so only the first matmul
                        # touching each pair-tile starts the group.
                        nc.tensor.matmul(
                            out=pscs[lqc // 2][
                                :, (lqc % 2) * J * E + j * E:
                                (lqc % 2) * J * E + (j + 1) * E],
                            lhsT=et[:, lqc * P:(lqc + 1) * P],
                            rhs=va_list[j][:, ci, h * E:(h + 1) * E],
                            start=(j == 0 and ci == 0 and lqc % 2 == 0),
                            stop=(j == J - 1 and ci == 3 and lqc % 2 == 1))

                # Pair the scores matmuls so the PE array keeps one tile
                # config (64x128) for two consecutive MMs, then run the 8
                # ctx MMs (128x128 config) of the previous pair. Mixed
                # configs back-to-back expose the PE pipeline drain.
                steps = [(j, ci) for j in range(J) for ci in range(4)]
                pend = []
                for si in range(0, 12, 2):
                    for (j, ci) in steps[si:si + 2]:
                        pss = ps_att.tile([P, LQ], F32, tag="att", name="ps_s")
                        nc.tensor.matmul(
                            out=pss[:],
                            lhsT=kT[j][ti][r0:r0 + HD, ci * P:(ci + 1) * P],
                            rhs=qT[ti][r0:r0 + HD, :], start=True, stop=True)
                        et = expp.tile([P, LQ], BF16, tag="expp", name="et")
                        nc.scalar.activation(
                            out=et[:], in_=pss[:],
                            func=mybir.ActivationFunctionType.Exp,
                            scale=1.0 / math.sqrt(HD))
                        if dbg and b == 0 and h == 0 and j == 0:
                            nc.sync.dma_start(out=aps["dbg_exp"][ci], in_=et[:])
                        pend.append((j, ci, et))
                    for _ in range(2):
                        if fillers:
                            fillers.pop(0)()
                    while len(pend) > 2:
                        emit_ctx(*pend.pop(0))
                while pend:
                    emit_ctx(*pend.pop(0))
                while fillers:
                    fillers.pop(0)()

                # normalize: denominator is column HD of each head stripe
                cat_tiles = []
                for pi in range(2):
                    rec = recp.tile([P, 2 * J], F32, tag="rec", name="rec")
                    nc.vector.reciprocal(
                        rec[:],
                        pscs[pi].rearrange("p (x e) -> p x e", e=E)[:, :, HD])
                    for half in range(2):
                        cat = catp.tile([P, J * HD], BF16, tag="cat",
                                        name="cat")
                        for j in range(J):
                            nc.vector.tensor_scalar_mul(
                                cat[:, j * HD:(j + 1) * HD],
                                pscs[pi][:, half * J * E + j * E:
                                         half * J * E + j * E + HD],
                                rec[:, half * J + j:half * J + j + 1])
                        cat_tiles.append(cat)
                    if dbg and b == 0 and h == 0:
                        nc.sync.dma_start(out=aps["dbg_rec"][pi], in_=rec[:])
                cat_all[h] = cat_tiles
                fillers = emit_transposes(h, cat_tiles)
            while fillers:
                fillers.pop(0)()
            if dbg and b == 0 and hg == 0:
                for lqc in range(4):
                    nc.sync.dma_start(out=aps["dbg_cat"][lqc],
                                      in_=cat_all[0][lqc][:])

            # ---- MLP1 / Gelu / MLP2 for the 8-head group, head pairs ----
            for h in range(hg * HG, (hg + 1) * HG, 2):
                hs = (h, h + 1)
                ph1s = {}
                for w_t, cidx, st in ((w1a, 0, True), (w1b, 1, False)):
                    for hh in hs:
                        if st:
                            ph1s[hh] = ps_big.tile([P, LQ], F32, tag="big",
                                                   name="ph1")
                        nc.tensor.matmul(
                            out=ph1s[hh][0:HD, :], lhsT=w_t[:],
                            rhs=catT_all[hh][cidx][:],
                            start=st, stop=not st)
                h1s = {}
                for hh in hs:
                    h1 = h1p.tile([HD, LQ], BF16, tag="h1", name="h1")
                    nc.scalar.activation(
                        out=h1[:], in_=ph1s[hh][0:HD, :],
                        func=mybir.ActivationFunctionType.Gelu, bias=b1_sb[:])
                    h1s[hh] = h1
                if dbg and b == 0 and h == 0:
                    nc.sync.dma_start(out=aps["dbg_catT0"][:, :],
                                      in_=catT_all[0][0][:])
                    nc.sync.dma_start(out=aps["dbg_h1"][:, :], in_=h1s[0][:])
                for hh in hs:
                    for lqc in range(4):
                        ps2 = ps_att.tile([P, LQ], F32, tag="att", name="ps2")
                        nc.tensor.matmul(
                            out=ps2[:, 0:HD],
                            lhsT=h1s[hh][:, lqc * P:(lqc + 1) * P],
                            rhs=w2t[:], start=True, stop=True)
                        nc.vector.tensor_add(
                            ost[lqc][:, hh * HD:(hh + 1) * HD], ps2[:, 0:HD],
                            b2_bc[:, hh * HD:(hh + 1) * HD])

        for lqc in range(4):
            nc.sync.dma_start(out=aps["out"][b, lqc * P:(lqc + 1) * P, :],
                              in_=ost[lqc][:])

    for p in reversed(ctx_mgr):
        p.__exit__(None, None, None)


_CACHE = {}


def _build(dbg=False):
    key = ("nc", dbg)
    if key in _CACHE:
        return _CACHE[key]
    nc = bacc.Bacc("TRN2", target_bir_lowering=False, debug=False)
    shapes = {
        "qt_in": ([B_LOC, D, LQ], BF16),
        "kt_in": ([J, B_LOC, D, LK], BF16),
        "vt_in": ([J, B_LOC, D, LK], BF16),
        "wqt": ([D, D], BF16),
        "wkt": ([D, D], BF16),
        "wvt": ([D, D], BF16),
        "w1t": ([J * HD, HD], BF16),
        "w2t": ([HD, HD], BF16),
        "ident": ([P, P], BF16),
        "ones_cols": ([P, 4, H], BF16),
        "bq": ([P, 8], F32),
        "bk": ([P, 8], F32),
        "bv_bc": ([P, D], BF16),
        "b2_bc": ([P, D], F32),
        "b1": ([HD, 1], F32),
    }
    aps = {k: nc.dram_tensor(k, s, dt, kind="ExternalInput").ap()
           for k, (s, dt) in shapes.items()}
    aps["out"] = nc.dram_tensor("out", [B_LOC, LQ, D], F32,
                                kind="ExternalOutput").ap()
    if dbg:
        dbg_shapes = {
            "dbg_qt": ([8, P, 512], BF16), "dbg_kt0": ([8, P, 512], BF16),
            "dbg_va0": ([P, 4, H * E], BF16), "dbg_exp": ([4, P, LQ], BF16),
            "dbg_rec": ([2, P, 2 * J], F32), "dbg_cat": ([4, P, J * HD], BF16),
            "dbg_catT0": ([P, LQ], BF16), "dbg_h1": ([HD, LQ], BF16),
        }
        for k, (shp, dt) in dbg_shapes.items():
            aps[k] = nc.dram_tensor(k, shp, dt, kind="ExternalOutput").ap()
    with tile.TileContext(nc) as tc:
        _emit(tc, aps, dbg=dbg)
    nc.compile()
    _CACHE[key] = nc
    return nc


def _prep_in_maps(inputs):
    f32 = np.float32
    bf16 = ml_dtypes.bfloat16
    q = np.ascontiguousarray(np.asarray(inputs["query_states"], f32))
    k = np.ascontiguousarray(np.asarray(inputs["key_states"], f32))
    v = np.ascontiguousarray(np.asarray(inputs["value_states"], f32))
    Wq = np.asarray(inputs["Wq"], f32)
    Wk = np.asarray(inputs["Wk"], f32)
    Wv = np.asarray(inputs["Wv"], f32)
    W1 = np.asarray(inputs["W1"], f32)
    W2 = np.asarray(inputs["W2"], f32)
    bq = np.asarray(inputs["bq"], f32)
    bk = np.asarray(inputs["bk"], f32)
    bv = np.asarray(inputs["bv"], f32)
    b1 = np.asarray(inputs["b1"], f32)
    b2 = np.asarray(inputs["b2"], f32)

    wqt = np.ascontiguousarray(Wq.T).astype(bf16)
    wkt = np.ascontiguousarray(Wk.T).astype(bf16)
    wvt = np.ascontiguousarray(Wv.T).astype(bf16)
    w1t = np.ascontiguousarray(W1.T).astype(bf16)          # [192, 64]
    w2t = np.ascontiguousarray(W2.T).astype(bf16)          # [64, 64]
    ident = np.eye(P, dtype=f32).astype(bf16)
    bq_sb = np.ascontiguousarray(bq.reshape(8, P).T).astype(f32)
    bk_sb = np.ascontiguousarray(bk.reshape(8, P).T).astype(f32)
    bv_bc = np.tile(bv, (P, 1)).astype(bf16)
    b2_bc = np.tile(b2, (P, H)).astype(f32)
    b1_col = b1.reshape(HD, 1).astype(f32)
    ones_cols = np.ones((P, 4, H), f32).astype(bf16)

    qt_all = np.ascontiguousarray(q.transpose(0, 2, 1)).astype(bf16)
    kt_all = np.ascontiguousarray(k.transpose(0, 1, 3, 2)).astype(bf16)
    vt_all = np.ascontiguousarray(v.transpose(0, 1, 3, 2)).astype(bf16)

    in_maps = []
    for c in range(N_CORES):
        sl = slice(c * B_LOC, (c + 1) * B_LOC)
        in_maps.append({
            "qt_in": np.ascontiguousarray(qt_all[sl]),
            "kt_in": np.ascontiguousarray(kt_all[:, sl]),
            "vt_in": np.ascontiguousarray(vt_all[:, sl]),
            "wqt": wqt, "wkt": wkt, "wvt": wvt,
            "w1t": w1t, "w2t": w2t, "ident": ident,
            "ones_cols": ones_cols,
            "bq": bq_sb, "bk": bk_sb, "bv_bc": bv_bc,
            "b2_bc": b2_bc, "b1": b1_col,
        })
    return in_maps


def kernel(**inputs):
    nc = _build()
    in_maps = _prep_in_maps(inputs)
    res = run_bass_kernel_spmd(nc, in_maps, core_ids=list(range(N_CORES)))
    out = np.concatenate([res.results[i]["out"] for i in range(N_CORES)], axis=0)
    return out.astype(np.float32)


# revision 29
# speedup vs baseline: 2.6661x; 1.2314x over previous
"""Trainium2 Bass kernel for nn_MeshCrossAttention (mesh cross-attention + per-head MLP).

Sharding: data-parallel over batch B=16 -> 2 batches per NeuronCore, 8 cores,
no collectives.

v2 design (vs v1 baseline at ~1.33 ms):
  - bf16 operands everywhere on the PE (fp32 PSUM accumulate). Halves DMA and
    SBUF traffic; all projection weights stay RESIDENT in SBUF (loaded once).
  - Transposed projections exactly like v1 (qT/kT via lhsT=W^T chunks), V in
    natural head-interleaved layout va [LK, 4, H*(HD+1)] with a ones column.
  - Scores stay transposed (sT [LK, LQ]; lhsT = kT head slice), exp on ScalarE
    -> eT bf16 tiles.
  - Context is accumulated in NATURAL layout: ctx[LQ, j*(HD+1)] via
    lhsT = eT chunk [LK, LQ-chunk], rhs = va slice [LK, HD+1]. The ones column
    of va makes column HD the softmax denominator, which now lives PER
    PARTITION -> normalization is a plain DVE reciprocal + tensor_scalar
    multiply. No DRAM-roundtrip partition broadcast (v1's big serializer).
  - cat [LQ, 192] is transposed back with PE identity-matmuls for the per-head
    MLP (contraction over 192 needs cat^T), then MLP1 -> Gelu -> MLP2 with the
    MLP2 output written naturally into a [LQ, D] staging tile -> single DMA out.
  - Exp and Gelu are batched in 8-head phases so the ScalarE activation table
    swaps 4x per batch instead of 52x.
  - PSUM budget: big(2) + att(2) + ctx(3, paired [128, 390]) + m64(1) = 8 banks.
"""
import math
import sys

import numpy as np

if "/opt/trn_rl_repo" not in sys.path:
    sys.path.insert(0, "/opt/trn_rl_repo")

import ml_dtypes  # noqa: E402

import concourse.bass as bass  # noqa: E402
import concourse.tile as tile  # noqa: E402
from concourse import bacc, mybir  # noqa: E402
from concourse.bass_utils import run_bass_kernel_spmd  # noqa: E402

F32 = mybir.dt.float32
BF16 = mybir.dt.bfloat16

D, H, HD, J = 1024, 16, 64, 3
B, LQ, LK = 16, 512, 512
P = 128
N_CORES = 8
B_LOC = B // N_CORES  # 2
E = HD + 1            # 65: head stripe width in va (ones column at HD)
HG = 8                # heads per exp/gelu phase group


def _emit(tc, aps, dbg=False):
    nc = tc.nc
    ctx_mgr = []

    def pool(name, bufs, space="SBUF"):
        p = tc.tile_pool(name=name, bufs=bufs, space=space)
        ctx_mgr.append(p)
        return p.__enter__()

    const = pool("const", 1)
    ain = pool("ain", 10)          # streamed activation chunks [128, 512] bf16
    qt_pool = pool("qt", 16)       # qT tiles, double-buffered across batches
    va_pool = pool("va", 3)
    expp = pool("expp", 8)
    recp = pool("recp", 4)
    catp = pool("catp", 26)        # pair cat tiles, consumed by fillers
    ctp = pool("ctp", 14)          # catTj pair tiles, alive until mini-MLP
    h1p = pool("h1p", 4)
    ostg = pool("ostg", 5)

    ps_big = pool("ps_big", 2, "PSUM")   # proj accum + cat transposes + mlp1
    ps_att = pool("ps_att", 3, "PSUM")   # scores / mlp2 [128, 512]
    ps_ctx = pool("ps_ctx", 3, "PSUM")   # ctx pairs [128, 2*J*E = 390]

    # ---------------- resident constants ----------------
    wq_sb, wk_sb, wv_sb = [], [], []
    for nm, lst in (("wqt", wq_sb), ("wkt", wk_sb), ("wvt", wv_sb)):
        for i in range(8):
            t = const.tile([P, D], BF16, tag=f"{nm}{i}", name=f"{nm}{i}")
            nc.sync.dma_start(out=t[:], in_=aps[nm][i * P:(i + 1) * P, :])
            lst.append(t)

    w1jd = []
    for j in range(J):
        t = const.tile([P, P], BF16, tag=f"w1jd{j}", name=f"w1jd{j}")
        nc.sync.dma_start(out=t[:], in_=aps["w1jd"][j])
        w1jd.append(t)
    w2bd = const.tile([P, P], BF16, tag="w2bd", name="w2bd")
    nc.sync.dma_start(out=w2bd[:], in_=aps["w2bd"][:, :])
    ident = const.tile([P, P], BF16, tag="ident", name="ident")
    nc.sync.dma_start(out=ident[:], in_=aps["ident"][:, :])

    bq_sb = const.tile([P, 8], F32, tag="bq", name="bq_sb")
    nc.sync.dma_start(out=bq_sb[:], in_=aps["bq"][:, :])
    bk_sb = const.tile([P, 8], F32, tag="bk", name="bk_sb")
    nc.sync.dma_start(out=bk_sb[:], in_=aps["bk"][:, :])
    bv_bc = const.tile([P, D], BF16, tag="bv", name="bv_bc")
    nc.sync.dma_start(out=bv_bc[:], in_=aps["bv_bc"][:, :])
    b2_bc = const.tile([P, D], F32, tag="b2", name="b2_bc")
    nc.sync.dma_start(out=b2_bc[:], in_=aps["b2_bc"][:, :])
    b1_sb = const.tile([P, 1], F32, tag="b1", name="b1_sb")
    nc.sync.dma_start(out=b1_sb[:], in_=aps["b1"][:, :])

    # kT in zero-padded per-head layout: tile [128, 512] with only this
    # head's 64 rows live, the other 64 rows ZERO. Scores then run as full
    # 128x128x512 matmuls (partial-K matmuls measure ~2x slower on HW); the
    # zero rows multiply the paired head's q rows to nothing. Tiles persist
    # (const pool) so the zeros are written once.
    ktp = [[const.tile([P, LK], BF16, tag=f"ktp{j}_{h}", name=f"ktp{j}_{h}")
            for h in range(H)] for j in range(J)]
    for j in range(J):
        for h in range(H):
            pad = slice(HD, P) if h % 2 == 0 else slice(0, HD)
            nc.gpsimd.memset(ktp[j][h][pad, :], 0.0)

    def load_acts(ap_slice):
        ts = []
        for ic in range(8):
            t = ain.tile([P, 512], BF16, tag="ain", name="act")
            nc.sync.dma_start(out=t[:], in_=ap_slice[ic * P:(ic + 1) * P, :])
            ts.append(t)
        return ts

    def proj_T(w_tiles, x_tiles, bias_sb, out_pool, out_tag):
        """out[oc] [128, 512] = (W @ x^T) chunk + bias, bf16."""
        outs = []
        for oc in range(8):
            pss = ps_big.tile([P, 512], F32, tag="big", name="pss")
            for ic in range(8):
                nc.tensor.matmul(
                    out=pss[:], lhsT=w_tiles[ic][:, oc * P:(oc + 1) * P],
                    rhs=x_tiles[ic][:], start=(ic == 0), stop=(ic == 7))
            t = out_pool.tile([P, 512], BF16, tag=out_tag, name=out_tag)
            nc.vector.tensor_scalar_add(t[:], pss[:], bias_sb[:, oc:oc + 1])
            outs.append(t)
        return outs

    def proj_K(x_tiles, bias_sb, j):
        """K projection into the zero-padded per-head layout."""
        for oc in range(8):
            pss = ps_big.tile([P, 512], F32, tag="big", name="pssk")
            for ic in range(8):
                nc.tensor.matmul(
                    out=pss[:], lhsT=wk_sb[ic][:, oc * P:(oc + 1) * P],
                    rhs=x_tiles[ic][:], start=(ic == 0), stop=(ic == 7))
            nc.vector.tensor_scalar_add(
                ktp[j][2 * oc][0:HD, :], pss[0:HD, :],
                bias_sb[0:HD, oc:oc + 1])
            nc.vector.tensor_scalar_add(
                ktp[j][2 * oc + 1][HD:P, :], pss[HD:P, :],
                bias_sb[HD:P, oc:oc + 1])

    def proj_V(x_tiles, va):
        """va [128, 4, H*E] natural head-interleaved V + ones column."""
        nc.sync.dma_start(
            out=va.rearrange("p c (h e) -> p c h e", e=E)[:, :, :, HD],
            in_=aps["ones_cols"][:, :, :])
        for half in range(2):
            for nck in range(4):
                pss = ps_big.tile([P, 512], F32, tag="big", name="pssv")
                for ic in range(8):
                    nc.tensor.matmul(
                        out=pss[:],
                        lhsT=x_tiles[ic][:, nck * P:(nck + 1) * P],
                        rhs=wv_sb[ic][:, half * 512:(half + 1) * 512],
                        start=(ic == 0), stop=(ic == 7))
                dst = va[:, nck, :].rearrange("p (h e) -> p h e", e=E)[
                    :, half * 8:(half + 1) * 8, 0:HD]
                nc.vector.tensor_tensor(
                    out=dst,
                    in0=pss[:].rearrange("p (h e) -> p h e", e=HD),
                    in1=bv_bc[:, half * 512:(half + 1) * 512].rearrange(
                        "p (h e) -> p h e", e=HD),
                    op=mybir.AluOpType.add)

    for b in range(B_LOC):
        # ================= projections =================
        qin = load_acts(aps["qt_in"][b])
        qT = proj_T(wq_sb, qin, bq_sb, qt_pool, "qt")

        for j in range(J):
            kin = load_acts(aps["kt_in"][j, b])
            proj_K(kin, bk_sb, j)

        va_list = []
        for j in range(J):
            vin = load_acts(aps["vt_in"][j, b])
            va = va_pool.tile([P, 4, H * E], BF16, tag="va", name="va")
            proj_V(vin, va)
            va_list.append(va)

        if dbg and b == 0:
            for oc in range(8):
                nc.sync.dma_start(out=aps["dbg_qt"][oc], in_=qT[oc][:])
                nc.sync.dma_start(out=aps["dbg_kt0"][oc], in_=ktp[0][oc][:])
            nc.sync.dma_start(out=aps["dbg_va0"][:, :, :], in_=va_list[0][:])

        ost = [ostg.tile([P, D], F32, tag="ostg", name=f"ost{i}")
               for i in range(4)]

        # ================= attention + MLP, 8-head phases =================
        # Per head: 12 (score -> exp) steps; ctx matmuls consume the PREVIOUS
        # step's exp output so the PE never waits on ScalarE. The previous
        # head's cat transposes are interleaved as PE filler during exp
        # latency. Every PE matmul in this phase is a full 128x128 tile
        # (partial-K/M matmuls measure ~2x slower on HW): scores use the
        # zero-padded kT, the j2 cat columns are packed per head PAIR and
        # MLP1/MLP2 use block-diagonal weights over head pairs.
        for hg in range(H // HG):
            catT_all = {}
            catp_tiles = None
            fillers = []
            for h in range(hg * HG, (hg + 1) * HG):
                ti = h // 2
                pscs = [ps_ctx.tile([P, 2 * J * E], F32, tag="ctx",
                                    name=f"psc{i}") for i in range(2)]

                def emit_ctx(j, ci, et):
                    for lqc in range(4):
                        # One start/stop per PSUM bank: start lazily zeroes
                        # the whole 2KB bank, so only the first matmul
                        # touching each pair-tile starts the group.
                        nc.tensor.matmul(
                            out=pscs[lqc // 2][
                                :, (lqc % 2) * J * E + j * E:
                                (lqc % 2) * J * E + (j + 1) * E],
                            lhsT=et[:, lqc * P:(lqc + 1) * P],
                            rhs=va_list[j][:, ci, h * E:(h + 1) * E],
                            start=(j == 0 and ci == 0 and lqc % 2 == 0),
                            stop=(j == J - 1 and ci == 3 and lqc % 2 == 1))

                prev = None
                for j in range(J):
                    for ci in range(4):
                        pss = ps_att.tile([P, LQ], F32, tag="att", name="ps_s")
                        nc.tensor.matmul(
                            out=pss[:],
                            lhsT=ktp[j][h][:, ci * P:(ci + 1) * P],
                            rhs=qT[ti][:], start=True, stop=True)
                        et = expp.tile([P, LQ], BF16, tag="expp", name="et")
                        nc.scalar.activation(
                            out=et[:], in_=pss[:],
                            func=mybir.ActivationFunctionType.Exp,
                            scale=1.0 / math.sqrt(HD))
                        if dbg and b == 0 and h == 0 and j == 0:
                            nc.sync.dma_start(out=aps["dbg_exp"][ci], in_=et[:])
                        if fillers:
                            fillers.pop(0)()
                        if prev is not None:
                            emit_ctx(*prev)
                        prev = (j, ci, et)
                emit_ctx(*prev)

                # normalize: denominator is column HD of each head stripe.
                # Each j-chunk goes into the head PAIR's shared catp tile
                # (64 columns per head) so the transposes and MLP1 run as
                # full 128x128 tiles with block-diagonal weights.
                if h % 2 == 0:
                    catp_tiles = [[catp.tile([P, P], BF16, tag="catp",
                                             name="catp")
                                   for _ in range(4)] for _ in range(J)]
                for pi in range(2):
                    rec = recp.tile([P, 2 * J], F32, tag="rec", name="rec")
                    nc.vector.reciprocal(
                        rec[:],
                        pscs[pi].rearrange("p (x e) -> p x e", e=E)[:, :, HD])
                    for half in range(2):
                        lqc = pi * 2 + half
                        for j in range(J):
                            nc.vector.tensor_scalar_mul(
                                catp_tiles[j][lqc][:, (h % 2) * HD:
                                                   (h % 2) * HD + HD],
                                pscs[pi][:, half * J * E + j * E:
                                         half * J * E + j * E + HD],
                                rec[:, half * J + j:half * J + j + 1])
                    if dbg and b == 0 and h == 0:
                        nc.sync.dma_start(out=aps["dbg_rec"][pi], in_=rec[:])
                if dbg and b == 0 and h == 1:
                    for lqc in range(4):
                        nc.sync.dma_start(out=aps["dbg_cat"][lqc],
                                          in_=catp_tiles[0][lqc][:])

                # After the odd head, queue the pair's transpose work as PE
                # filler for the next head's exp-latency slots.
                if h % 2 == 1:
                    thunks = []
                    catTs = []
                    for j in range(J):
                        ptj = ps_big.tile([P, LQ], F32, tag="big", name="ptj")
                        catTj = ctp.tile([P, LQ], BF16, tag="ct", name="catTj")
                        for lqc in range(4):
                            thunks.append(
                                lambda lqc=lqc, ptj=ptj, c=catp_tiles[j][lqc]:
                                nc.tensor.matmul(
                                    out=ptj[:, lqc * P:(lqc + 1) * P],
                                    lhsT=c[:], rhs=ident[:],
                                    start=(lqc == 0), stop=(lqc == 3)))
                        thunks.append(
                            lambda ptj=ptj, catTj=catTj:
                            nc.vector.tensor_copy(out=catTj[:], in_=ptj[:]))
                        catTs.append(catTj)
                    catT_all[h // 2] = catTs
                    fillers.extend(thunks)
            while fillers:
                fillers.pop(0)()

            # ---- MLP1 / Gelu / MLP2 per head pair, full 128-tiles ----
            for pi in range(hg * HG // 2, (hg + 1) * HG // 2):
                ph1p = ps_big.tile([P, LQ], F32, tag="big", name="ph1p")
                for j in range(J):
                    nc.tensor.matmul(out=ph1p[:, :], lhsT=w1jd[j][:],
                                     rhs=catT_all[pi][j][:],
                                     start=(j == 0), stop=(j == J - 1))
                h1 = h1p.tile([P, LQ], BF16, tag="h1", name="h1")
                nc.scalar.activation(
                    out=h1[:], in_=ph1p[:],
                    func=mybir.ActivationFunctionType.Gelu, bias=b1_sb[:])
                if dbg and b == 0 and pi == 0:
                    nc.sync.dma_start(out=aps["dbg_catT0"][:, :],
                                      in_=catT_all[0][0][:])
                    nc.sync.dma_start(out=aps["dbg_h1"][:, :], in_=h1[:])
                for lqc in range(4):
                    ps2 = ps_att.tile([P, LQ], F32, tag="att", name="ps2")
                    nc.tensor.matmul(
                        out=ps2[:, 0:P], lhsT=h1[:, lqc * P:(lqc + 1) * P],
                        rhs=w2bd[:], start=True, stop=True)
                    nc.vector.tensor_add(
                        ost[lqc][:, pi * P:(pi + 1) * P], ps2[:, 0:P],
                        b2_bc[:, pi * P:(pi + 1) * P])

        for lqc in range(4):
            nc.sync.dma_start(out=aps["out"][b, lqc * P:(lqc + 1) * P, :],
                              in_=ost[lqc][:])

    for p in reversed(ctx_mgr):
        p.__exit__(None, None, None)


_CACHE = {}


def _build(dbg=False):
    key = ("nc", dbg)
    if key in _CACHE:
        return _CACHE[key]
    nc = bacc.Bacc("TRN2", target_bir_lowering=False, debug=False)
    shapes = {
        "qt_in": ([B_LOC, D, LQ], BF16),
        "kt_in": ([J, B_LOC, D, LK], BF16),
        "vt_in": ([J, B_LOC, D, LK], BF16),
        "wqt": ([D, D], BF16),
        "wkt": ([D, D], BF16),
        "wvt": ([D, D], BF16),
        "w1jd": ([J, P, P], BF16),
        "w2bd": ([P, P], BF16),
        "ident": ([P, P], BF16),
        "ones_cols": ([P, 4, H], BF16),
        "bq": ([P, 8], F32),
        "bk": ([P, 8], F32),
        "bv_bc": ([P, D], BF16),
        "b2_bc": ([P, D], F32),
        "b1": ([P, 1], F32),
    }
    aps = {k: nc.dram_tensor(k, s, dt, kind="ExternalInput").ap()
           for k, (s, dt) in shapes.items()}
    aps["out"] = nc.dram_tensor("out", [B_LOC, LQ, D], F32,
                                kind="ExternalOutput").ap()
    if dbg:
        dbg_shapes = {
            "dbg_qt": ([8, P, 512], BF16), "dbg_kt0": ([8, P, 512], BF16),
            "dbg_va0": ([P, 4, H * E], BF16), "dbg_exp": ([4, P, LQ], BF16),
            "dbg_rec": ([2, P, 2 * J], F32), "dbg_cat": ([4, P, P], BF16),
            "dbg_catT0": ([P, LQ], BF16), "dbg_h1": ([P, LQ], BF16),
        }
        for k, (shp, dt) in dbg_shapes.items():
            aps[k] = nc.dram_tensor(k, shp, dt, kind="ExternalOutput").ap()
    with tile.TileContext(nc) as tc:
        _emit(tc, aps, dbg=dbg)
    nc.compile()
    _CACHE[key] = nc
    return nc


def _prep_in_maps(inputs):
    f32 = np.float32
    bf16 = ml_dtypes.bfloat16
    q = np.ascontiguousarray(np.asarray(inputs["query_states"], f32))
    k = np.ascontiguousarray(np.asarray(inputs["key_states"], f32))
    v = np.ascontiguousarray(np.asarray(inputs["value_states"], f32))
    Wq = np.asarray(inputs["Wq"], f32)
    Wk = np.asarray(inputs["Wk"], f32)
    Wv = np.asarray(inputs["Wv"], f32)
    W1 = np.asarray(inputs["W1"], f32)
    W2 = np.asarray(inputs["W2"], f32)
    bq = np.asarray(inputs["bq"], f32)
    bk = np.asarray(inputs["bk"], f32)
    bv = np.asarray(inputs["bv"], f32)
    b1 = np.asarray(inputs["b1"], f32)
    b2 = np.asarray(inputs["b2"], f32)

    wqt = np.ascontiguousarray(Wq.T).astype(bf16)
    wkt = np.ascontiguousarray(Wk.T).astype(bf16)
    wvt = np.ascontiguousarray(Wv.T).astype(bf16)
    W1T = np.ascontiguousarray(W1.T)                       # [192, 64]
    w1jd = np.zeros((J, P, P), f32)
    for j in range(J):
        blk = W1T[j * HD:(j + 1) * HD]                     # [64, 64]
        w1jd[j, :HD, :HD] = blk
        w1jd[j, HD:, HD:] = blk
    w1jd = w1jd.astype(bf16)
    W2T = np.ascontiguousarray(W2.T)                       # [64, 64]
    w2bd = np.zeros((P, P), f32)
    w2bd[:HD, :HD] = W2T
    w2bd[HD:, HD:] = W2T
    w2bd = w2bd.astype(bf16)
    ident = np.eye(P, dtype=f32).astype(bf16)
    bq_sb = np.ascontiguousarray(bq.reshape(8, P).T).astype(f32)
    bk_sb = np.ascontiguousarray(bk.reshape(8, P).T).astype(f32)
    bv_bc = np.tile(bv, (P, 1)).astype(bf16)
    b2_bc = np.tile(b2, (P, H)).astype(f32)
    b1_col = np.concatenate([b1, b1]).reshape(P, 1).astype(f32)
    ones_cols = np.ones((P, 4, H), f32).astype(bf16)

    qt_all = np.ascontiguousarray(q.transpose(0, 2, 1)).astype(bf16)
    kt_all = np.ascontiguousarray(k.transpose(0, 1, 3, 2)).astype(bf16)
    vt_all = np.ascontiguousarray(v.transpose(0, 1, 3, 2)).astype(bf16)

    in_maps = []
    for c in range(N_CORES):
        sl = slice(c * B_LOC, (c + 1) * B_LOC)
        in_maps.append({
            "qt_in": np.ascontiguousarray(qt_all[sl]),
            "kt_in": np.ascontiguousarray(kt_all[:, sl]),
            "vt_in": np.ascontiguousarray(vt_all[:, sl]),
            "wqt": wqt, "wkt": wkt, "wvt": wvt,
            "w1jd": w1jd, "w2bd": w2bd, "ident": ident,
            "ones_cols": ones_cols,
            "bq": bq_sb, "bk": bk_sb, "bv_bc": bv_bc,
            "b2_bc": b2_bc, "b1": b1_col,
        })
    return in_maps


def kernel(**inputs):
    nc = _build()
    in_maps = _prep_in_maps(inputs)
    res = run_bass_kernel_spmd(nc, in_maps, core_ids=list(range(N_CORES)))
    out = np.concatenate([res.results[i]["out"] for i in range(N_CORES)], axis=0)
    return out.astype(np.float32)


# revision 30
# speedup vs baseline: 2.8012x; 1.0507x over previous
"""Trainium2 Bass kernel for nn_MeshCrossAttention (mesh cross-attention + per-head MLP).

Sharding: data-parallel over batch B=16 -> 2 batches per NeuronCore, 8 cores,
no collectives.

v2 design (vs v1 baseline at ~1.33 ms):
  - bf16 operands everywhere on the PE (fp32 PSUM accumulate). Halves DMA and
    SBUF traffic; all projection weights stay RESIDENT in SBUF (loaded once).
  - Transposed projections exactly like v1 (qT/kT via lhsT=W^T chunks), V in
    natural head-interleaved layout va [LK, 4, H*(HD+1)] with a ones column.
  - Scores stay transposed (sT [LK, LQ]; lhsT = kT head slice), exp on ScalarE
    -> eT bf16 tiles.
  - Context is accumulated in NATURAL layout: ctx[LQ, j*(HD+1)] via
    lhsT = eT chunk [LK, LQ-chunk], rhs = va slice [LK, HD+1]. The ones column
    of va makes column HD the softmax denominator, which now lives PER
    PARTITION -> normalization is a plain DVE reciprocal + tensor_scalar
    multiply. No DRAM-roundtrip partition broadcast (v1's big serializer).
  - cat [LQ, 192] is transposed back with PE identity-matmuls for the per-head
    MLP (contraction over 192 needs cat^T), then MLP1 -> Gelu -> MLP2 with the
    MLP2 output written naturally into a [LQ, D] staging tile -> single DMA out.
  - Exp and Gelu are batched in 8-head phases so the ScalarE activation table
    swaps 4x per batch instead of 52x.
  - PSUM budget: big(2) + att(2) + ctx(3, paired [128, 390]) + m64(1) = 8 banks.
"""
import math
import sys

import numpy as np

if "/opt/trn_rl_repo" not in sys.path:
    sys.path.insert(0, "/opt/trn_rl_repo")

import ml_dtypes  # noqa: E402

import concourse.bass as bass  # noqa: E402
import concourse.tile as tile  # noqa: E402
from concourse import bacc, mybir  # noqa: E402
from concourse.bass_utils import run_bass_kernel_spmd  # noqa: E402

F32 = mybir.dt.float32
BF16 = mybir.dt.bfloat16

D, H, HD, J = 1024, 16, 64, 3
B, LQ, LK = 16, 512, 512
P = 128
N_CORES = 8
B_LOC = B // N_CORES  # 2
E = HD + 1            # 65: head stripe width in va (ones column at HD)
HG = 8                # heads per exp/gelu phase group


def _emit(tc, aps, dbg=False):
    nc = tc.nc
    ctx_mgr = []

    def pool(name, bufs, space="SBUF"):
        p = tc.tile_pool(name=name, bufs=bufs, space=space)
        ctx_mgr.append(p)
        return p.__enter__()

    const = pool("const", 1)
    ain = pool("ain", 10)          # streamed activation chunks [128, 512] bf16
    qt_pool = pool("qt", 24)       # zero-padded per-head qT tiles [128, 512]
    kt_pool = pool("kt", 24)
    va_pool = pool("va", 3)
    expp = pool("expp", 8)
    recp = pool("recp", 4)
    catp = pool("catp", 26)        # pair cat tiles, consumed by fillers
    ctp = pool("ctp", 14)          # catTj pair tiles, alive until mini-MLP
    h1p = pool("h1p", 4)
    ostg = pool("ostg", 5)

    ps_big = pool("ps_big", 2, "PSUM")   # proj accum + cat transposes + mlp1
    ps_att = pool("ps_att", 3, "PSUM")   # scores / mlp2 [128, 512]
    ps_ctx = pool("ps_ctx", 3, "PSUM")   # ctx pairs [128, 2*J*E = 390]

    # ---------------- resident constants ----------------
    # Only wq/bq are DMAed up front (the first projection needs them); the
    # rest is deferred until after the Q-projection is emitted so the PE can
    # start ~8us earlier at kernel start.
    wq_sb, wk_sb, wv_sb = [], [], []
    for nm, lst in (("wqt", wq_sb), ("wkt", wk_sb), ("wvt", wv_sb)):
        for i in range(8):
            t = const.tile([P, D], BF16, tag=f"{nm}{i}", name=f"{nm}{i}")
            lst.append(t)
    for i in range(8):
        nc.sync.dma_start(out=wq_sb[i][:], in_=aps["wqt"][i * P:(i + 1) * P, :])
    bq_sb = const.tile([P, 8], F32, tag="bq", name="bq_sb")
    nc.sync.dma_start(out=bq_sb[:], in_=aps["bq"][:, :])

    w1jd = [const.tile([P, P], BF16, tag=f"w1jd{j}", name=f"w1jd{j}")
            for j in range(J)]
    w2bd = const.tile([P, P], BF16, tag="w2bd", name="w2bd")
    ident = const.tile([P, P], BF16, tag="ident", name="ident")
    bk_sb = const.tile([P, 8], F32, tag="bk", name="bk_sb")
    bv_bc = const.tile([P, D], BF16, tag="bv", name="bv_bc")
    b2_bc = const.tile([P, D], F32, tag="b2", name="b2_bc")
    b1_sb = const.tile([P, 1], F32, tag="b1", name="b1_sb")

    def deferred_const_dmas():
        for i in range(8):
            nc.sync.dma_start(out=wk_sb[i][:],
                              in_=aps["wkt"][i * P:(i + 1) * P, :])
        nc.sync.dma_start(out=bk_sb[:], in_=aps["bk"][:, :])
        for i in range(8):
            nc.sync.dma_start(out=wv_sb[i][:],
                              in_=aps["wvt"][i * P:(i + 1) * P, :])
        nc.sync.dma_start(out=bv_bc[:], in_=aps["bv_bc"][:, :])
        for j in range(J):
            nc.sync.dma_start(out=w1jd[j][:], in_=aps["w1jd"][j])
        nc.sync.dma_start(out=w2bd[:], in_=aps["w2bd"][:, :])
        nc.sync.dma_start(out=ident[:], in_=aps["ident"][:, :])
        nc.sync.dma_start(out=b2_bc[:], in_=aps["b2_bc"][:, :])
        nc.sync.dma_start(out=b1_sb[:], in_=aps["b1"][:, :])

    def load_acts(ap_slice):
        ts = []
        for ic in range(8):
            t = ain.tile([P, 512], BF16, tag="ain", name="act")
            nc.sync.dma_start(out=t[:], in_=ap_slice[ic * P:(ic + 1) * P, :])
            ts.append(t)
        return ts

    def proj_T(w_tiles, x_tiles, bias_sb, out_pool, out_tag):
        """out[oc] [128, 512] = (W @ x^T) chunk + bias, bf16."""
        outs = []
        for oc in range(8):
            pss = ps_big.tile([P, 512], F32, tag="big", name="pss")
            for ic in range(8):
                nc.tensor.matmul(
                    out=pss[:], lhsT=w_tiles[ic][:, oc * P:(oc + 1) * P],
                    rhs=x_tiles[ic][:], start=(ic == 0), stop=(ic == 7))
            t = out_pool.tile([P, 512], BF16, tag=out_tag, name=out_tag)
            nc.vector.tensor_scalar_add(t[:], pss[:], bias_sb[:, oc:oc + 1])
            outs.append(t)
        return outs

    def proj_Q(x_tiles):
        """Q projection into zero-padded per-head tiles qtp[h] [128, 512]:
        head h's 64 q-dims at rows (h%2)*64, the other 64 rows ZERO, so the
        scores matmul is a full 128x128x512 tile against the compact kT
        (the zero q rows null the paired head's k contribution)."""
        outs = []
        for oc in range(8):
            pss = ps_big.tile([P, 512], F32, tag="big", name="pss")
            for ic in range(8):
                nc.tensor.matmul(
                    out=pss[:], lhsT=wq_sb[ic][:, oc * P:(oc + 1) * P],
                    rhs=x_tiles[ic][:], start=(ic == 0), stop=(ic == 7))
            te = qt_pool.tile([P, 512], BF16, tag="qt", name="qtp_e")
            to = qt_pool.tile([P, 512], BF16, tag="qt", name="qtp_o")
            nc.gpsimd.memset(te[HD:P, :], 0.0)
            nc.gpsimd.memset(to[0:HD, :], 0.0)
            nc.vector.tensor_scalar_add(te[0:HD, :], pss[0:HD, :],
                                        bq_sb[0:HD, oc:oc + 1])
            nc.vector.tensor_scalar_add(to[HD:P, :], pss[HD:P, :],
                                        bq_sb[HD:P, oc:oc + 1])
            outs += [te, to]
        return outs

    def proj_V(x_tiles, va):
        """va [128, 4, H*E] natural head-interleaved V + ones column."""
        nc.sync.dma_start(
            out=va.rearrange("p c (h e) -> p c h e", e=E)[:, :, :, HD],
            in_=aps["ones_cols"][:, :, :])
        for half in range(2):
            for nck in range(4):
                pss = ps_big.tile([P, 512], F32, tag="big", name="pssv")
                for ic in range(8):
                    nc.tensor.matmul(
                        out=pss[:],
                        lhsT=x_tiles[ic][:, nck * P:(nck + 1) * P],
                        rhs=wv_sb[ic][:, half * 512:(half + 1) * 512],
                        start=(ic == 0), stop=(ic == 7))
                dst = va[:, nck, :].rearrange("p (h e) -> p h e", e=E)[
                    :, half * 8:(half + 1) * 8, 0:HD]
                nc.vector.tensor_tensor(
                    out=dst,
                    in0=pss[:].rearrange("p (h e) -> p h e", e=HD),
                    in1=bv_bc[:, half * 512:(half + 1) * 512].rearrange(
                        "p (h e) -> p h e", e=HD),
                    op=mybir.AluOpType.add)

    for b in range(B_LOC):
        # ================= projections =================
        qin = load_acts(aps["qt_in"][b])
        qtp = proj_Q(qin)
        if b == 0:
            deferred_const_dmas()

        kT = []
        for j in range(J):
            kin = load_acts(aps["kt_in"][j, b])
            kT.append(proj_T(wk_sb, kin, bk_sb, kt_pool, "kt"))

        va_list = []
        for j in range(J):
            vin = load_acts(aps["vt_in"][j, b])
            va = va_pool.tile([P, 4, H * E], BF16, tag="va", name="va")
            proj_V(vin, va)
            va_list.append(va)

        if dbg and b == 0:
            for oc in range(8):
                nc.sync.dma_start(out=aps["dbg_qt"][oc], in_=qtp[oc][:])
                nc.sync.dma_start(out=aps["dbg_kt0"][oc], in_=kT[0][oc][:])
            nc.sync.dma_start(out=aps["dbg_va0"][:, :, :], in_=va_list[0][:])

        ost = [ostg.tile([P, D], F32, tag="ostg", name=f"ost{i}")
               for i in range(4)]

        # ================= attention + MLP, 8-head phases =================
        # Per head: 12 (score -> exp) steps; ctx matmuls consume the PREVIOUS
        # step's exp output so the PE never waits on ScalarE. The previous
        # head's cat transposes are interleaved as PE filler during exp
        # latency. Every PE matmul in this phase is a full 128x128 tile
        # (partial-K/M matmuls measure ~2x slower on HW): scores use the
        # zero-padded kT, the j2 cat columns are packed per head PAIR and
        # MLP1/MLP2 use block-diagonal weights over head pairs.
        for hg in range(H // HG):
            catT_all = {}
            catp_tiles = None
            fillers = []
            for h in range(hg * HG, (hg + 1) * HG):
                pscs = [ps_ctx.tile([P, 2 * J * E], F32, tag="ctx",
                                    name=f"psc{i}") for i in range(2)]

                def emit_ctx(j, ci, et):
                    for lqc in range(4):
                        # One start/stop per PSUM bank: start lazily zeroes
                        # the whole 2KB bank, so only the first matmul
                        # touching each pair-tile starts the group.
                        nc.tensor.matmul(
                            out=pscs[lqc // 2][
                                :, (lqc % 2) * J * E + j * E:
                                (lqc % 2) * J * E + (j + 1) * E],
                            lhsT=et[:, lqc * P:(lqc + 1) * P],
                            rhs=va_list[j][:, ci, h * E:(h + 1) * E],
                            start=(j == 0 and ci == 0 and lqc % 2 == 0),
                            stop=(j == J - 1 and ci == 3 and lqc % 2 == 1))

                prev = None
                for j in range(J):
                    for ci in range(4):
                        pss = ps_att.tile([P, LQ], F32, tag="att", name="ps_s")
                        nc.tensor.matmul(
                            out=pss[:],
                            lhsT=kT[j][h // 2][:, ci * P:(ci + 1) * P],
                            rhs=qtp[h][:], start=True, stop=True)
                        et = expp.tile([P, LQ], BF16, tag="expp", name="et")
                        nc.scalar.activation(
                            out=et[:], in_=pss[:],
                            func=mybir.ActivationFunctionType.Exp,
                            scale=1.0 / math.sqrt(HD))
                        if dbg and b == 0 and h == 0 and j == 0:
                            nc.sync.dma_start(out=aps["dbg_exp"][ci], in_=et[:])
                        if fillers:
                            fillers.pop(0)()
                        if prev is not None:
                            emit_ctx(*prev)
                        prev = (j, ci, et)
                emit_ctx(*prev)

                # normalize: denominator is column HD of each head stripe.
                # Each j-chunk goes into the head PAIR's shared catp tile
                # (64 columns per head) so the transposes and MLP1 run as
                # full 128x128 tiles with block-diagonal weights.
                if h % 2 == 0:
                    catp_tiles = [[catp.tile([P, P], BF16, tag="catp",
                                             name="catp")
                                   for _ in range(4)] for _ in range(J)]
                for pi in range(2):
                    rec = recp.tile([P, 2 * J], F32, tag="rec", name="rec")
                    nc.vector.reciprocal(
                        rec[:],
                        pscs[pi].rearrange("p (x e) -> p x e", e=E)[:, :, HD])
                    for half in range(2):
                        lqc = pi * 2 + half
                        for j in range(J):
                            nc.vector.tensor_scalar_mul(
                                catp_tiles[j][lqc][:, (h % 2) * HD:
                                                   (h % 2) * HD + HD],
                                pscs[pi][:, half * J * E + j * E:
                                         half * J * E + j * E + HD],
                                rec[:, half * J + j:half * J + j + 1])
                    if dbg and b == 0 and h == 0:
                        nc.sync.dma_start(out=aps["dbg_rec"][pi], in_=rec[:])
                if dbg and b == 0 and h == 1:
                    for lqc in range(4):
                        nc.sync.dma_start(out=aps["dbg_cat"][lqc],
                                          in_=catp_tiles[0][lqc][:])

                # After the odd head, queue the pair's transpose work as PE
                # filler for the next head's exp-latency slots.
                if h % 2 == 1:
                    thunks = []
                    catTs = []
                    for j in range(J):
                        ptj = ps_big.tile([P, LQ], F32, tag="big", name="ptj")
                        catTj = ctp.tile([P, LQ], BF16, tag="ct", name="catTj")
                        for lqc in range(4):
                            thunks.append(
                                lambda lqc=lqc, ptj=ptj, c=catp_tiles[j][lqc]:
                                nc.tensor.matmul(
                                    out=ptj[:, lqc * P:(lqc + 1) * P],
                                    lhsT=c[:], rhs=ident[:],
                                    start=(lqc == 0), stop=(lqc == 3)))
                        thunks.append(
                            lambda ptj=ptj, catTj=catTj:
                            nc.vector.tensor_copy(out=catTj[:], in_=ptj[:]))
                        catTs.append(catTj)
                    catT_all[h // 2] = catTs
                    fillers.extend(thunks)
            while fillers:
                fillers.pop(0)()

            # ---- MLP1 / Gelu / MLP2 per head pair, full 128-tiles ----
            for pi in range(hg * HG // 2, (hg + 1) * HG // 2):
                ph1p = ps_big.tile([P, LQ], F32, tag="big", name="ph1p")
                for j in range(J):
                    nc.tensor.matmul(out=ph1p[:, :], lhsT=w1jd[j][:],
                                     rhs=catT_all[pi][j][:],
                                     start=(j == 0), stop=(j == J - 1))
                h1 = h1p.tile([P, LQ], BF16, tag="h1", name="h1")
                nc.scalar.activation(
                    out=h1[:], in_=ph1p[:],
                    func=mybir.ActivationFunctionType.Gelu, bias=b1_sb[:])
                if dbg and b == 0 and pi == 0:
                    nc.sync.dma_start(out=aps["dbg_catT0"][:, :],
                                      in_=catT_all[0][0][:])
                    nc.sync.dma_start(out=aps["dbg_h1"][:, :], in_=h1[:])
                for lqc in range(4):
                    ps2 = ps_att.tile([P, LQ], F32, tag="att", name="ps2")
                    nc.tensor.matmul(
                        out=ps2[:, 0:P], lhsT=h1[:, lqc * P:(lqc + 1) * P],
                        rhs=w2bd[:], start=True, stop=True)
                    nc.vector.tensor_add(
                        ost[lqc][:, pi * P:(pi + 1) * P], ps2[:, 0:P],
                        b2_bc[:, pi * P:(pi + 1) * P])
            # stream out this head-group's half of the output columns
            for lqc in range(4):
                nc.sync.dma_start(
                    out=aps["out"][b, lqc * P:(lqc + 1) * P,
                                   hg * 512:(hg + 1) * 512],
                    in_=ost[lqc][:, hg * 512:(hg + 1) * 512])


    for p in reversed(ctx_mgr):
        p.__exit__(None, None, None)


_CACHE = {}


def _build(dbg=False):
    key = ("nc", dbg)
    if key in _CACHE:
        return _CACHE[key]
    nc = bacc.Bacc("TRN2", target_bir_lowering=False, debug=False)
    shapes = {
        "qt_in": ([B_LOC, D, LQ], BF16),
        "kt_in": ([J, B_LOC, D, LK], BF16),
        "vt_in": ([J, B_LOC, D, LK], BF16),
        "wqt": ([D, D], BF16),
        "wkt": ([D, D], BF16),
        "wvt": ([D, D], BF16),
        "w1jd": ([J, P, P], BF16),
        "w2bd": ([P, P], BF16),
        "ident": ([P, P], BF16),
        "ones_cols": ([P, 4, H], BF16),
        "bq": ([P, 8], F32),
        "bk": ([P, 8], F32),
        "bv_bc": ([P, D], BF16),
        "b2_bc": ([P, D], F32),
        "b1": ([P, 1], F32),
    }
    aps = {k: nc.dram_tensor(k, s, dt, kind="ExternalInput").ap()
           for k, (s, dt) in shapes.items()}
    aps["out"] = nc.dram_tensor("out", [B_LOC, LQ, D], F32,
                                kind="ExternalOutput").ap()
    if dbg:
        dbg_shapes = {
            "dbg_qt": ([8, P, 512], BF16), "dbg_kt0": ([8, P, 512], BF16),
            "dbg_va0": ([P, 4, H * E], BF16), "dbg_exp": ([4, P, LQ], BF16),
            "dbg_rec": ([2, P, 2 * J], F32), "dbg_cat": ([4, P, P], BF16),
            "dbg_catT0": ([P, LQ], BF16), "dbg_h1": ([P, LQ], BF16),
        }
        for k, (shp, dt) in dbg_shapes.items():
            aps[k] = nc.dram_tensor(k, shp, dt, kind="ExternalOutput").ap()
    with tile.TileContext(nc) as tc:
        _emit(tc, aps, dbg=dbg)
    nc.compile()
    _CACHE[key] = nc
    return nc


def _prep_in_maps(inputs):
    f32 = np.float32
    bf16 = ml_dtypes.bfloat16
    q = np.ascontiguousarray(np.asarray(inputs["query_states"], f32))
    k = np.ascontiguousarray(np.asarray(inputs["key_states"], f32))
    v = np.ascontiguousarray(np.asarray(inputs["value_states"], f32))
    Wq = np.asarray(inputs["Wq"], f32)
    Wk = np.asarray(inputs["Wk"], f32)
    Wv = np.asarray(inputs["Wv"], f32)
    W1 = np.asarray(inputs["W1"], f32)
    W2 = np.asarray(inputs["W2"], f32)
    bq = np.asarray(inputs["bq"], f32)
    bk = np.asarray(inputs["bk"], f32)
    bv = np.asarray(inputs["bv"], f32)
    b1 = np.asarray(inputs["b1"], f32)
    b2 = np.asarray(inputs["b2"], f32)

    wqt = np.ascontiguousarray(Wq.T).astype(bf16)
    wkt = np.ascontiguousarray(Wk.T).astype(bf16)
    wvt = np.ascontiguousarray(Wv.T).astype(bf16)
    W1T = np.ascontiguousarray(W1.T)                       # [192, 64]
    w1jd = np.zeros((J, P, P), f32)
    for j in range(J):
        blk = W1T[j * HD:(j + 1) * HD]                     # [64, 64]
        w1jd[j, :HD, :HD] = blk
        w1jd[j, HD:, HD:] = blk
    w1jd = w1jd.astype(bf16)
    W2T = np.ascontiguousarray(W2.T)                       # [64, 64]
    w2bd = np.zeros((P, P), f32)
    w2bd[:HD, :HD] = W2T
    w2bd[HD:, HD:] = W2T
    w2bd = w2bd.astype(bf16)
    ident = np.eye(P, dtype=f32).astype(bf16)
    bq_sb = np.ascontiguousarray(bq.reshape(8, P).T).astype(f32)
    bk_sb = np.ascontiguousarray(bk.reshape(8, P).T).astype(f32)
    bv_bc = np.tile(bv, (P, 1)).astype(bf16)
    b2_bc = np.tile(b2, (P, H)).astype(f32)
    b1_col = np.concatenate([b1, b1]).reshape(P, 1).astype(f32)
    ones_cols = np.ones((P, 4, H), f32).astype(bf16)

    qt_all = np.ascontiguousarray(q.transpose(0, 2, 1)).astype(bf16)
    kt_all = np.ascontiguousarray(k.transpose(0, 1, 3, 2)).astype(bf16)
    vt_all = np.ascontiguousarray(v.transpose(0, 1, 3, 2)).astype(bf16)

    in_maps = []
    for c in range(N_CORES):
        sl = slice(c * B_LOC, (c + 1) * B_LOC)
        in_maps.append({
            "qt_in": np.ascontiguousarray(qt_all[sl]),
            "kt_in": np.ascontiguousarray(kt_all[:, sl]),
            "vt_in": np.ascontiguousarray(vt_all[:, sl]),
            "wqt": wqt, "wkt": wkt, "wvt": wvt,
            "w1jd": w1jd, "w2bd": w2bd, "ident": ident,
            "ones_cols": ones_cols,
            "bq": bq_sb, "bk": bk_sb, "bv_bc": bv_bc,
            "b2_bc": b2_bc, "b1": b1_col,
        })
    return in_maps


def kernel(**inputs):
    nc = _build()
    in_maps = _prep_in_maps(inputs)
    res = run_bass_kernel_spmd(nc, in_maps, core_ids=list(range(N_CORES)))
    out = np.concatenate([res.results[i]["out"] for i in range(N_CORES)], axis=0)
    return out.astype(np.float32)


# revision 31
# speedup vs baseline: 2.8709x; 1.0249x over previous
"""Trainium2 Bass kernel for nn_MeshCrossAttention (mesh cross-attention + per-head MLP).

Sharding: data-parallel over batch B=16 -> 2 batches per NeuronCore, 8 cores,
no collectives.

v2 design (vs v1 baseline at ~1.33 ms):
  - bf16 operands everywhere on the PE (fp32 PSUM accumulate). Halves DMA and
    SBUF traffic; all projection weights stay RESIDENT in SBUF (loaded once).
  - Transposed projections exactly like v1 (qT/kT via lhsT=W^T chunks), V in
    natural head-interleaved layout va [LK, 4, H*(HD+1)] with a ones column.
  - Scores stay transposed (sT [LK, LQ]; lhsT = kT head slice), exp on ScalarE
    -> eT bf16 tiles.
  - Context is accumulated in NATURAL layout: ctx[LQ, j*(HD+1)] via
    lhsT = eT chunk [LK, LQ-chunk], rhs = va slice [LK, HD+1]. The ones column
    of va makes column HD the softmax denominator, which now lives PER
    PARTITION -> normalization is a plain DVE reciprocal + tensor_scalar
    multiply. No DRAM-roundtrip partition broadcast (v1's big serializer).
  - cat [LQ, 192] is transposed back with PE identity-matmuls for the per-head
    MLP (contraction over 192 needs cat^T), then MLP1 -> Gelu -> MLP2 with the
    MLP2 output written naturally into a [LQ, D] staging tile -> single DMA out.
  - Exp and Gelu are batched in 8-head phases so the ScalarE activation table
    swaps 4x per batch instead of 52x.
  - PSUM budget: big(2) + att(2) + ctx(3, paired [128, 390]) + m64(1) = 8 banks.
"""
import math
import sys

import numpy as np

if "/opt/trn_rl_repo" not in sys.path:
    sys.path.insert(0, "/opt/trn_rl_repo")

import ml_dtypes  # noqa: E402

import concourse.bass as bass  # noqa: E402
import concourse.tile as tile  # noqa: E402
from concourse import bacc, mybir  # noqa: E402
from concourse.bass_utils import run_bass_kernel_spmd  # noqa: E402

F32 = mybir.dt.float32
BF16 = mybir.dt.bfloat16

D, H, HD, J = 1024, 16, 64, 3
B, LQ, LK = 16, 512, 512
P = 128
N_CORES = 8
B_LOC = B // N_CORES  # 2
E = HD + 1            # 65: head stripe width in va (ones column at HD)
HG = 8                # heads per exp/gelu phase group


def _emit(tc, aps, dbg=False):
    nc = tc.nc
    ctx_mgr = []

    def pool(name, bufs, space="SBUF"):
        p = tc.tile_pool(name=name, bufs=bufs, space=space)
        ctx_mgr.append(p)
        return p.__enter__()

    const = pool("const", 1)
    ain = pool("ain", 12)          # streamed activation chunks [128, 512] bf16
    qt_pool = pool("qt", 24)       # zero-padded per-head qT tiles [128, 512]
    kt_pool = pool("kt", 24)
    va_pool = pool("va", 3)
    expp = pool("expp", 8)
    recp = pool("recp", 4)
    catp = pool("catp", 10)        # pair cat tiles [128, 384]
    ctp = pool("ctp", 14)          # catTj pair tiles, alive until mini-MLP
    h1p = pool("h1p", 4)
    ostg = pool("ostg", 5)

    ps_big = pool("ps_big", 2, "PSUM")   # proj accum + cat transposes + mlp1
    ps_att = pool("ps_att", 3, "PSUM")   # scores / mlp2 [128, 512]
    ps_ctx = pool("ps_ctx", 3, "PSUM")   # ctx pairs [128, 2*J*E = 390]

    # ---------------- resident constants ----------------
    # Only wq/bq are DMAed up front (the first projection needs them); the
    # rest is deferred until after the Q-projection is emitted so the PE can
    # start ~8us earlier at kernel start.
    wq_sb, wk_sb, wv_sb = [], [], []
    for nm, lst in (("wqt", wq_sb), ("wkt", wk_sb), ("wvt", wv_sb)):
        for i in range(8):
            t = const.tile([P, D], BF16, tag=f"{nm}{i}", name=f"{nm}{i}")
            lst.append(t)
    for i in range(8):
        nc.sync.dma_start(out=wq_sb[i][:], in_=aps["wqt"][i * P:(i + 1) * P, :])
    bq_sb = const.tile([P, 8], F32, tag="bq", name="bq_sb")
    nc.sync.dma_start(out=bq_sb[:], in_=aps["bq"][:, :])

    w1jd = [const.tile([P, P], BF16, tag=f"w1jd{j}", name=f"w1jd{j}")
            for j in range(J)]
    w2bd = const.tile([P, P], BF16, tag="w2bd", name="w2bd")
    ident = const.tile([P, P], BF16, tag="ident", name="ident")
    bk_sb = const.tile([P, 8], F32, tag="bk", name="bk_sb")
    bv_bc = const.tile([P, D], BF16, tag="bv", name="bv_bc")
    b2_bc = const.tile([P, D], F32, tag="b2", name="b2_bc")
    b1_sb = const.tile([P, 1], F32, tag="b1", name="b1_sb")

    def deferred_const_dmas():
        for i in range(8):
            nc.sync.dma_start(out=wk_sb[i][:],
                              in_=aps["wkt"][i * P:(i + 1) * P, :])
        nc.sync.dma_start(out=bk_sb[:], in_=aps["bk"][:, :])
        for i in range(8):
            nc.sync.dma_start(out=wv_sb[i][:],
                              in_=aps["wvt"][i * P:(i + 1) * P, :])
        nc.sync.dma_start(out=bv_bc[:], in_=aps["bv_bc"][:, :])
        for j in range(J):
            nc.sync.dma_start(out=w1jd[j][:], in_=aps["w1jd"][j])
        nc.sync.dma_start(out=w2bd[:], in_=aps["w2bd"][:, :])
        nc.sync.dma_start(out=ident[:], in_=aps["ident"][:, :])
        nc.sync.dma_start(out=b2_bc[:], in_=aps["b2_bc"][:, :])
        nc.sync.dma_start(out=b1_sb[:], in_=aps["b1"][:, :])

    def load_acts(ap_slice):
        ts = []
        for ic in range(8):
            t = ain.tile([P, 512], BF16, tag="ain", name="act")
            nc.sync.dma_start(out=t[:], in_=ap_slice[ic * P:(ic + 1) * P, :])
            ts.append(t)
        return ts

    def proj_T(w_tiles, x_tiles, bias_sb, out_pool, out_tag):
        """out[oc] [128, 512] = (W @ x^T) chunk + bias, bf16."""
        outs = []
        for oc in range(8):
            pss = ps_big.tile([P, 512], F32, tag="big", name="pss")
            for ic in range(8):
                nc.tensor.matmul(
                    out=pss[:], lhsT=w_tiles[ic][:, oc * P:(oc + 1) * P],
                    rhs=x_tiles[ic][:], start=(ic == 0), stop=(ic == 7))
            t = out_pool.tile([P, 512], BF16, tag=out_tag, name=out_tag)
            nc.vector.tensor_scalar_add(t[:], pss[:], bias_sb[:, oc:oc + 1])
            outs.append(t)
        return outs

    def proj_Q(x_tiles):
        """Q projection into zero-padded per-head tiles qtp[h] [128, 512]:
        head h's 64 q-dims at rows (h%2)*64, the other 64 rows ZERO, so the
        scores matmul is a full 128x128x512 tile against the compact kT
        (the zero q rows null the paired head's k contribution)."""
        outs = []
        for oc in range(8):
            pss = ps_big.tile([P, 512], F32, tag="big", name="pss")
            for ic in range(8):
                nc.tensor.matmul(
                    out=pss[:], lhsT=wq_sb[ic][:, oc * P:(oc + 1) * P],
                    rhs=x_tiles[ic][:], start=(ic == 0), stop=(ic == 7))
            te = qt_pool.tile([P, 512], BF16, tag="qt", name="qtp_e")
            to = qt_pool.tile([P, 512], BF16, tag="qt", name="qtp_o")
            nc.gpsimd.memset(te[HD:P, :], 0.0)
            nc.gpsimd.memset(to[0:HD, :], 0.0)
            nc.vector.tensor_scalar_add(te[0:HD, :], pss[0:HD, :],
                                        bq_sb[0:HD, oc:oc + 1])
            nc.vector.tensor_scalar_add(to[HD:P, :], pss[HD:P, :],
                                        bq_sb[HD:P, oc:oc + 1])
            outs += [te, to]
        return outs

    def proj_V(x_tiles, va):
        """va [128, 4, H*E] natural head-interleaved V + ones column."""
        nc.sync.dma_start(
            out=va.rearrange("p c (h e) -> p c h e", e=E)[:, :, :, HD],
            in_=aps["ones_cols"][:, :, :])
        for half in range(2):
            for nck in range(4):
                pss = ps_big.tile([P, 512], F32, tag="big", name="pssv")
                for ic in range(8):
                    nc.tensor.matmul(
                        out=pss[:],
                        lhsT=x_tiles[ic][:, nck * P:(nck + 1) * P],
                        rhs=wv_sb[ic][:, half * 512:(half + 1) * 512],
                        start=(ic == 0), stop=(ic == 7))
                dst = va[:, nck, :].rearrange("p (h e) -> p h e", e=E)[
                    :, half * 8:(half + 1) * 8, 0:HD]
                nc.vector.tensor_tensor(
                    out=dst,
                    in0=pss[:].rearrange("p (h e) -> p h e", e=HD),
                    in1=bv_bc[:, half * 512:(half + 1) * 512].rearrange(
                        "p (h e) -> p h e", e=HD),
                    op=mybir.AluOpType.add)

    for b in range(B_LOC):
        # ================= projections =================
        qin = load_acts(aps["qt_in"][b])
        qtp = proj_Q(qin)
        if b == 0:
            deferred_const_dmas()

        kT = []
        for j in range(J):
            kin = load_acts(aps["kt_in"][j, b])
            kT.append(proj_T(wk_sb, kin, bk_sb, kt_pool, "kt"))

        va_list = []
        for j in range(J):
            vin = load_acts(aps["vt_in"][j, b])
            va = va_pool.tile([P, 4, H * E], BF16, tag="va", name="va")
            proj_V(vin, va)
            va_list.append(va)

        if dbg and b == 0:
            for oc in range(8):
                nc.sync.dma_start(out=aps["dbg_qt"][oc], in_=qtp[oc][:])
                nc.sync.dma_start(out=aps["dbg_kt0"][oc], in_=kT[0][oc][:])
            nc.sync.dma_start(out=aps["dbg_va0"][:, :, :], in_=va_list[0][:])

        ost = [ostg.tile([P, D], F32, tag="ostg", name=f"ost{i}")
               for i in range(4)]

        # ================= attention + MLP, 8-head phases =================
        # Per head: 12 (score -> exp) steps; ctx matmuls consume the PREVIOUS
        # step's exp output so the PE never waits on ScalarE. The previous
        # head's cat transposes are interleaved as PE filler during exp
        # latency. Every PE matmul in this phase is a full 128x128 tile
        # (partial-K/M matmuls measure ~2x slower on HW): scores use the
        # zero-padded kT, the j2 cat columns are packed per head PAIR and
        # MLP1/MLP2 use block-diagonal weights over head pairs.
        for hg in range(H // HG):
            catT_all = {}
            catp_tiles = None
            fillers = []
            for h in range(hg * HG, (hg + 1) * HG):
                pscs = [ps_ctx.tile([P, 2 * J * E], F32, tag="ctx",
                                    name=f"psc{i}") for i in range(2)]

                def emit_ctx(j, ci, et):
                    for lqc in range(4):
                        # One start/stop per PSUM bank: start lazily zeroes
                        # the whole 2KB bank, so only the first matmul
                        # touching each pair-tile starts the group.
                        nc.tensor.matmul(
                            out=pscs[lqc // 2][
                                :, (lqc % 2) * J * E + j * E:
                                (lqc % 2) * J * E + (j + 1) * E],
                            lhsT=et[:, lqc * P:(lqc + 1) * P],
                            rhs=va_list[j][:, ci, h * E:(h + 1) * E],
                            start=(j == 0 and ci == 0 and lqc % 2 == 0),
                            stop=(j == J - 1 and ci == 3 and lqc % 2 == 1))

                pend = []
                for j in range(J):
                    for ci in range(4):
                        pss = ps_att.tile([P, LQ], F32, tag="att", name="ps_s")
                        nc.tensor.matmul(
                            out=pss[:],
                            lhsT=kT[j][h // 2][:, ci * P:(ci + 1) * P],
                            rhs=qtp[h][:], start=True, stop=True)
                        et = expp.tile([P, LQ], BF16, tag="expp", name="et")
                        nc.scalar.activation(
                            out=et[:], in_=pss[:],
                            func=mybir.ActivationFunctionType.Exp,
                            scale=1.0 / math.sqrt(HD))
                        if dbg and b == 0 and h == 0 and j == 0:
                            nc.sync.dma_start(out=aps["dbg_exp"][ci], in_=et[:])
                        if fillers:
                            fillers.pop(0)()
                        pend.append((j, ci, et))
                        if len(pend) > 2:
                            emit_ctx(*pend.pop(0))
                while pend:
                    emit_ctx(*pend.pop(0))

                # normalize: denominator is column HD of each head stripe.
                # Each j-chunk goes into the head PAIR's shared catp tile
                # (64 columns per head) so the transposes and MLP1 run as
                # full 128x128 tiles with block-diagonal weights.
                if h % 2 == 0:
                    catp_tiles = [catp.tile([P, J * P], BF16, tag="catp",
                                            name="catp") for _ in range(4)]
                for pi in range(2):
                    rec = recp.tile([P, 2 * J], F32, tag="rec", name="rec")
                    nc.vector.reciprocal(
                        rec[:],
                        pscs[pi].rearrange("p (x e) -> p x e", e=E)[:, :, HD])
                    for half in range(2):
                        lqc = pi * 2 + half
                        # one fused multiply: [128, 3, 64] x per-(row,j)
                        # reciprocal broadcast along the last dim
                        nc.vector.tensor_tensor(
                            out=catp_tiles[lqc].rearrange(
                                "p (x e) -> p x e", e=P)[
                                :, :, (h % 2) * HD:(h % 2) * HD + HD],
                            in0=pscs[pi].rearrange(
                                "p (x e) -> p x e", e=E)[
                                :, half * J:(half + 1) * J, 0:HD],
                            in1=rec[:, half * J:(half + 1) * J].unsqueeze(2)
                            .to_broadcast((P, J, HD)),
                            op=mybir.AluOpType.mult)
                    if dbg and b == 0 and h == 0:
                        nc.sync.dma_start(out=aps["dbg_rec"][pi], in_=rec[:])
                if dbg and b == 0 and h == 1:
                    for lqc in range(4):
                        nc.sync.dma_start(out=aps["dbg_cat"][lqc],
                                          in_=catp_tiles[lqc][:, 0:P])

                # After the odd head, queue the pair's transpose work as PE
                # filler for the next head's exp-latency slots.
                if h % 2 == 1:
                    thunks = []
                    catTs = []
                    for j in range(J):
                        ptj = ps_big.tile([P, LQ], F32, tag="big", name="ptj")
                        catTj = ctp.tile([P, LQ], BF16, tag="ct", name="catTj")
                        for lqc in range(4):
                            thunks.append(
                                lambda lqc=lqc, ptj=ptj, j=j,
                                c=catp_tiles[lqc]:
                                nc.tensor.matmul(
                                    out=ptj[:, lqc * P:(lqc + 1) * P],
                                    lhsT=c[:, j * P:(j + 1) * P], rhs=ident[:],
                                    start=(lqc == 0), stop=(lqc == 3)))
                        thunks.append(
                            lambda ptj=ptj, catTj=catTj:
                            nc.vector.tensor_copy(out=catTj[:], in_=ptj[:]))
                        catTs.append(catTj)
                    catT_all[h // 2] = catTs
                    fillers.extend(thunks)
            while fillers:
                fillers.pop(0)()

            # ---- MLP1 / Gelu / MLP2 per head pair, full 128-tiles ----
            for pi in range(hg * HG // 2, (hg + 1) * HG // 2):
                ph1p = ps_big.tile([P, LQ], F32, tag="big", name="ph1p")
                for j in range(J):
                    nc.tensor.matmul(out=ph1p[:, :], lhsT=w1jd[j][:],
                                     rhs=catT_all[pi][j][:],
                                     start=(j == 0), stop=(j == J - 1))
                h1 = h1p.tile([P, LQ], BF16, tag="h1", name="h1")
                nc.scalar.activation(
                    out=h1[:], in_=ph1p[:],
                    func=mybir.ActivationFunctionType.Gelu, bias=b1_sb[:])
                if dbg and b == 0 and pi == 0:
                    nc.sync.dma_start(out=aps["dbg_catT0"][:, :],
                                      in_=catT_all[0][0][:])
                    nc.sync.dma_start(out=aps["dbg_h1"][:, :], in_=h1[:])
                for lqc in range(4):
                    ps2 = ps_att.tile([P, LQ], F32, tag="att", name="ps2")
                    nc.tensor.matmul(
                        out=ps2[:, 0:P], lhsT=h1[:, lqc * P:(lqc + 1) * P],
                        rhs=w2bd[:], start=True, stop=True)
                    nc.vector.tensor_add(
                        ost[lqc][:, pi * P:(pi + 1) * P], ps2[:, 0:P],
                        b2_bc[:, pi * P:(pi + 1) * P])
            # stream out this head-group's half of the output columns
            for lqc in range(4):
                nc.sync.dma_start(
                    out=aps["out"][b, lqc * P:(lqc + 1) * P,
                                   hg * 512:(hg + 1) * 512],
                    in_=ost[lqc][:, hg * 512:(hg + 1) * 512])


    for p in reversed(ctx_mgr):
        p.__exit__(None, None, None)


_CACHE = {}


def _build(dbg=False):
    key = ("nc", dbg)
    if key in _CACHE:
        return _CACHE[key]
    nc = bacc.Bacc("TRN2", target_bir_lowering=False, debug=False)
    shapes = {
        "qt_in": ([B_LOC, D, LQ], BF16),
        "kt_in": ([J, B_LOC, D, LK], BF16),
        "vt_in": ([J, B_LOC, D, LK], BF16),
        "wqt": ([D, D], BF16),
        "wkt": ([D, D], BF16),
        "wvt": ([D, D], BF16),
        "w1jd": ([J, P, P], BF16),
        "w2bd": ([P, P], BF16),
        "ident": ([P, P], BF16),
        "ones_cols": ([P, 4, H], BF16),
        "bq": ([P, 8], F32),
        "bk": ([P, 8], F32),
        "bv_bc": ([P, D], BF16),
        "b2_bc": ([P, D], F32),
        "b1": ([P, 1], F32),
    }
    aps = {k: nc.dram_tensor(k, s, dt, kind="ExternalInput").ap()
           for k, (s, dt) in shapes.items()}
    aps["out"] = nc.dram_tensor("out", [B_LOC, LQ, D], F32,
                                kind="ExternalOutput").ap()
    if dbg:
        dbg_shapes = {
            "dbg_qt": ([8, P, 512], BF16), "dbg_kt0": ([8, P, 512], BF16),
            "dbg_va0": ([P, 4, H * E], BF16), "dbg_exp": ([4, P, LQ], BF16),
            "dbg_rec": ([2, P, 2 * J], F32), "dbg_cat": ([4, P, P], BF16),
            "dbg_catT0": ([P, LQ], BF16), "dbg_h1": ([P, LQ], BF16),
        }
        for k, (shp, dt) in dbg_shapes.items():
            aps[k] = nc.dram_tensor(k, shp, dt, kind="ExternalOutput").ap()
    with tile.TileContext(nc) as tc:
        _emit(tc, aps, dbg=dbg)
    nc.compile()
    _CACHE[key] = nc
    return nc


def _prep_in_maps(inputs):
    f32 = np.float32
    bf16 = ml_dtypes.bfloat16
    q = np.ascontiguousarray(np.asarray(inputs["query_states"], f32))
    k = np.ascontiguousarray(np.asarray(inputs["key_states"], f32))
    v = np.ascontiguousarray(np.asarray(inputs["value_states"], f32))
    Wq = np.asarray(inputs["Wq"], f32)
    Wk = np.asarray(inputs["Wk"], f32)
    Wv = np.asarray(inputs["Wv"], f32)
    W1 = np.asarray(inputs["W1"], f32)
    W2 = np.asarray(inputs["W2"], f32)
    bq = np.asarray(inputs["bq"], f32)
    bk = np.asarray(inputs["bk"], f32)
    bv = np.asarray(inputs["bv"], f32)
    b1 = np.asarray(inputs["b1"], f32)
    b2 = np.asarray(inputs["b2"], f32)

    wqt = np.ascontiguousarray(Wq.T).astype(bf16)
    wkt = np.ascontiguousarray(Wk.T).astype(bf16)
    wvt = np.ascontiguousarray(Wv.T).astype(bf16)
    W1T = np.ascontiguousarray(W1.T)                       # [192, 64]
    w1jd = np.zeros((J, P, P), f32)
    for j in range(J):
        blk = W1T[j * HD:(j + 1) * HD]                     # [64, 64]
        w1jd[j, :HD, :HD] = blk
        w1jd[j, HD:, HD:] = blk
    w1jd = w1jd.astype(bf16)
    W2T = np.ascontiguousarray(W2.T)                       # [64, 64]
    w2bd = np.zeros((P, P), f32)
    w2bd[:HD, :HD] = W2T
    w2bd[HD:, HD:] = W2T
    w2bd = w2bd.astype(bf16)
    ident = np.eye(P, dtype=f32).astype(bf16)
    bq_sb = np.ascontiguousarray(bq.reshape(8, P).T).astype(f32)
    bk_sb = np.ascontiguousarray(bk.reshape(8, P).T).astype(f32)
    bv_bc = np.tile(bv, (P, 1)).astype(bf16)
    b2_bc = np.tile(b2, (P, H)).astype(f32)
    b1_col = np.concatenate([b1, b1]).reshape(P, 1).astype(f32)
    ones_cols = np.ones((P, 4, H), f32).astype(bf16)

    qt_all = np.ascontiguousarray(q.transpose(0, 2, 1)).astype(bf16)
    kt_all = np.ascontiguousarray(k.transpose(0, 1, 3, 2)).astype(bf16)
    vt_all = np.ascontiguousarray(v.transpose(0, 1, 3, 2)).astype(bf16)

    in_maps = []
    for c in range(N_CORES):
        sl = slice(c * B_LOC, (c + 1) * B_LOC)
        in_maps.append({
            "qt_in": np.ascontiguousarray(qt_all[sl]),
            "kt_in": np.ascontiguousarray(kt_all[:, sl]),
            "vt_in": np.ascontiguousarray(vt_all[:, sl]),
            "wqt": wqt, "wkt": wkt, "wvt": wvt,
            "w1jd": w1jd, "w2bd": w2bd, "ident": ident,
            "ones_cols": ones_cols,
            "bq": bq_sb, "bk": bk_sb, "bv_bc": bv_bc,
            "b2_bc": b2_bc, "b1": b1_col,
        })
    return in_maps


def kernel(**inputs):
    nc = _build()
    in_maps = _prep_in_maps(inputs)
    res = run_bass_kernel_spmd(nc, in_maps, core_ids=list(range(N_CORES)))
    out = np.concatenate([res.results[i]["out"] for i in range(N_CORES)], axis=0)
    return out.astype(np.float32)


# revision 32
# speedup vs baseline: 2.8779x; 1.0024x over previous
"""Trainium2 Bass kernel for nn_MeshCrossAttention (mesh cross-attention + per-head MLP).

Sharding: data-parallel over batch B=16 -> 2 batches per NeuronCore, 8 cores,
no collectives.

v2 design (vs v1 baseline at ~1.33 ms):
  - bf16 operands everywhere on the PE (fp32 PSUM accumulate). Halves DMA and
    SBUF traffic; all projection weights stay RESIDENT in SBUF (loaded once).
  - Transposed projections exactly like v1 (qT/kT via lhsT=W^T chunks), V in
    natural head-interleaved layout va [LK, 4, H*(HD+1)] with a ones column.
  - Scores stay transposed (sT [LK, LQ]; lhsT = kT head slice), exp on ScalarE
    -> eT bf16 tiles.
  - Context is accumulated in NATURAL layout: ctx[LQ, j*(HD+1)] via
    lhsT = eT chunk [LK, LQ-chunk], rhs = va slice [LK, HD+1]. The ones column
    of va makes column HD the softmax denominator, which now lives PER
    PARTITION -> normalization is a plain DVE reciprocal + tensor_scalar
    multiply. No DRAM-roundtrip partition broadcast (v1's big serializer).
  - cat [LQ, 192] is transposed back with PE identity-matmuls for the per-head
    MLP (contraction over 192 needs cat^T), then MLP1 -> Gelu -> MLP2 with the
    MLP2 output written naturally into a [LQ, D] staging tile -> single DMA out.
  - Exp and Gelu are batched in 8-head phases so the ScalarE activation table
    swaps 4x per batch instead of 52x.
  - PSUM budget: big(2) + att(2) + ctx(3, paired [128, 390]) + m64(1) = 8 banks.
"""
import math
import sys

import numpy as np

if "/opt/trn_rl_repo" not in sys.path:
    sys.path.insert(0, "/opt/trn_rl_repo")

import ml_dtypes  # noqa: E402

import concourse.bass as bass  # noqa: E402
import concourse.tile as tile  # noqa: E402
from concourse import bacc, mybir  # noqa: E402
from concourse.bass_utils import run_bass_kernel_spmd  # noqa: E402

F32 = mybir.dt.float32
BF16 = mybir.dt.bfloat16

D, H, HD, J = 1024, 16, 64, 3
B, LQ, LK = 16, 512, 512
P = 128
N_CORES = 8
B_LOC = B // N_CORES  # 2
E = HD + 1            # 65: head stripe width in va (ones column at HD)
HG = 8                # heads per exp/gelu phase group


def _emit(tc, aps, dbg=False):
    nc = tc.nc
    ctx_mgr = []

    def pool(name, bufs, space="SBUF"):
        p = tc.tile_pool(name=name, bufs=bufs, space=space)
        ctx_mgr.append(p)
        return p.__enter__()

    const = pool("const", 1)
    ain = pool("ain", 12)          # streamed activation chunks [128, 512] bf16
    qt_pool = pool("qt", 24)       # zero-padded per-head qT tiles [128, 512]
    kt_pool = pool("kt", 24)
    va_pool = pool("va", 3)
    expp = pool("expp", 10)
    recp = pool("recp", 4)
    catp = pool("catp", 10)        # pair cat tiles [128, 384]
    ctp = pool("ctp", 14)          # catTj pair tiles, alive until mini-MLP
    h1p = pool("h1p", 4)
    ostg = pool("ostg", 5)

    ps_big = pool("ps_big", 2, "PSUM")   # proj accum + cat transposes + mlp1
    ps_att = pool("ps_att", 3, "PSUM")   # scores / mlp2 [128, 512]
    ps_ctx = pool("ps_ctx", 3, "PSUM")   # ctx pairs [128, 2*J*E = 390]

    # ---------------- resident constants ----------------
    # Only wq/bq are DMAed up front (the first projection needs them); the
    # rest is deferred until after the Q-projection is emitted so the PE can
    # start ~8us earlier at kernel start.
    wq_sb, wk_sb, wv_sb = [], [], []
    for nm, lst in (("wqt", wq_sb), ("wkt", wk_sb), ("wvt", wv_sb)):
        for i in range(8):
            t = const.tile([P, D], BF16, tag=f"{nm}{i}", name=f"{nm}{i}")
            lst.append(t)
    for i in range(8):
        nc.sync.dma_start(out=wq_sb[i][:], in_=aps["wqt"][i * P:(i + 1) * P, :])
    bq_sb = const.tile([P, 8], F32, tag="bq", name="bq_sb")
    nc.sync.dma_start(out=bq_sb[:], in_=aps["bq"][:, :])

    w1jd = [const.tile([P, P], BF16, tag=f"w1jd{j}", name=f"w1jd{j}")
            for j in range(J)]
    w2bd = const.tile([P, P], BF16, tag="w2bd", name="w2bd")
    ident = const.tile([P, P], BF16, tag="ident", name="ident")
    bk_sb = const.tile([P, 8], F32, tag="bk", name="bk_sb")
    bv_bc = const.tile([P, D], BF16, tag="bv", name="bv_bc")
    b2_bc = const.tile([P, D], F32, tag="b2", name="b2_bc")
    b1_sb = const.tile([P, 1], F32, tag="b1", name="b1_sb")
    tbl_scr = const.tile([P, 1], F32, tag="tbl", name="tbl_scr")

    def deferred_const_dmas():
        for i in range(8):
            nc.sync.dma_start(out=wk_sb[i][:],
                              in_=aps["wkt"][i * P:(i + 1) * P, :])
        nc.sync.dma_start(out=bk_sb[:], in_=aps["bk"][:, :])
        for i in range(8):
            nc.sync.dma_start(out=wv_sb[i][:],
                              in_=aps["wvt"][i * P:(i + 1) * P, :])
        nc.sync.dma_start(out=bv_bc[:], in_=aps["bv_bc"][:, :])
        for j in range(J):
            nc.sync.dma_start(out=w1jd[j][:], in_=aps["w1jd"][j])
        nc.sync.dma_start(out=w2bd[:], in_=aps["w2bd"][:, :])
        nc.sync.dma_start(out=ident[:], in_=aps["ident"][:, :])
        nc.sync.dma_start(out=b2_bc[:], in_=aps["b2_bc"][:, :])
        nc.sync.dma_start(out=b1_sb[:], in_=aps["b1"][:, :])

    def load_acts(ap_slice):
        ts = []
        for ic in range(8):
            t = ain.tile([P, 512], BF16, tag="ain", name="act")
            nc.sync.dma_start(out=t[:], in_=ap_slice[ic * P:(ic + 1) * P, :])
            ts.append(t)
        return ts

    def proj_T(w_tiles, x_tiles, bias_sb, out_pool, out_tag):
        """out[oc] [128, 512] = (W @ x^T) chunk + bias, bf16."""
        outs = []
        for oc in range(8):
            pss = ps_big.tile([P, 512], F32, tag="big", name="pss")
            for ic in range(8):
                nc.tensor.matmul(
                    out=pss[:], lhsT=w_tiles[ic][:, oc * P:(oc + 1) * P],
                    rhs=x_tiles[ic][:], start=(ic == 0), stop=(ic == 7))
            t = out_pool.tile([P, 512], BF16, tag=out_tag, name=out_tag)
            nc.vector.tensor_scalar_add(t[:], pss[:], bias_sb[:, oc:oc + 1])
            outs.append(t)
        return outs

    def proj_Q(x_tiles):
        """Q projection into zero-padded per-head tiles qtp[h] [128, 512]:
        head h's 64 q-dims at rows (h%2)*64, the other 64 rows ZERO, so the
        scores matmul is a full 128x128x512 tile against the compact kT
        (the zero q rows null the paired head's k contribution)."""
        outs = []
        for oc in range(8):
            pss = ps_big.tile([P, 512], F32, tag="big", name="pss")
            for ic in range(8):
                nc.tensor.matmul(
                    out=pss[:], lhsT=wq_sb[ic][:, oc * P:(oc + 1) * P],
                    rhs=x_tiles[ic][:], start=(ic == 0), stop=(ic == 7))
            te = qt_pool.tile([P, 512], BF16, tag="qt", name="qtp_e")
            to = qt_pool.tile([P, 512], BF16, tag="qt", name="qtp_o")
            nc.gpsimd.memset(te[HD:P, :], 0.0)
            nc.gpsimd.memset(to[0:HD, :], 0.0)
            nc.vector.tensor_scalar_add(te[0:HD, :], pss[0:HD, :],
                                        bq_sb[0:HD, oc:oc + 1])
            nc.vector.tensor_scalar_add(to[HD:P, :], pss[HD:P, :],
                                        bq_sb[HD:P, oc:oc + 1])
            outs += [te, to]
        return outs

    def proj_V(x_tiles, va):
        """va [128, 4, H*E] natural head-interleaved V + ones column."""
        nc.sync.dma_start(
            out=va.rearrange("p c (h e) -> p c h e", e=E)[:, :, :, HD],
            in_=aps["ones_cols"][:, :, :])
        for half in range(2):
            for nck in range(4):
                pss = ps_big.tile([P, 512], F32, tag="big", name="pssv")
                for ic in range(8):
                    nc.tensor.matmul(
                        out=pss[:],
                        lhsT=x_tiles[ic][:, nck * P:(nck + 1) * P],
                        rhs=wv_sb[ic][:, half * 512:(half + 1) * 512],
                        start=(ic == 0), stop=(ic == 7))
                dst = va[:, nck, :].rearrange("p (h e) -> p h e", e=E)[
                    :, half * 8:(half + 1) * 8, 0:HD]
                nc.vector.tensor_tensor(
                    out=dst,
                    in0=pss[:].rearrange("p (h e) -> p h e", e=HD),
                    in1=bv_bc[:, half * 512:(half + 1) * 512].rearrange(
                        "p (h e) -> p h e", e=HD),
                    op=mybir.AluOpType.add)

    for b in range(B_LOC):
        # ================= projections =================
        qin = load_acts(aps["qt_in"][b])
        qtp = proj_Q(qin)
        if b == 0:
            deferred_const_dmas()

        kT = []
        for j in range(J):
            kin = load_acts(aps["kt_in"][j, b])
            kT.append(proj_T(wk_sb, kin, bk_sb, kt_pool, "kt"))

        va_list = []
        for j in range(J):
            vin = load_acts(aps["vt_in"][j, b])
            va = va_pool.tile([P, 4, H * E], BF16, tag="va", name="va")
            proj_V(vin, va)
            va_list.append(va)

        if dbg and b == 0:
            for oc in range(8):
                nc.sync.dma_start(out=aps["dbg_qt"][oc], in_=qtp[oc][:])
                nc.sync.dma_start(out=aps["dbg_kt0"][oc], in_=kT[0][oc][:])
            nc.sync.dma_start(out=aps["dbg_va0"][:, :, :], in_=va_list[0][:])

        ost = [ostg.tile([P, D], F32, tag="ostg", name=f"ost{i}")
               for i in range(4)]

        # ================= attention + MLP, 8-head phases =================
        # Per head: 12 (score -> exp) steps; ctx matmuls consume the PREVIOUS
        # step's exp output so the PE never waits on ScalarE. The previous
        # head's cat transposes are interleaved as PE filler during exp
        # latency. Every PE matmul in this phase is a full 128x128 tile
        # (partial-K/M matmuls measure ~2x slower on HW): scores use the
        # zero-padded kT, the j2 cat columns are packed per head PAIR and
        # MLP1/MLP2 use block-diagonal weights over head pairs.
        for hg in range(H // HG):
            catT_all = {}
            catp_tiles = None
            fillers = []
            for h in range(hg * HG, (hg + 1) * HG):
                pscs = [ps_ctx.tile([P, 2 * J * E], F32, tag="ctx",
                                    name=f"psc{i}") for i in range(2)]

                def emit_ctx(j, ci, et):
                    for lqc in range(4):
                        # One start/stop per PSUM bank: start lazily zeroes
                        # the whole 2KB bank, so only the first matmul
                        # touching each pair-tile starts the group.
                        nc.tensor.matmul(
                            out=pscs[lqc // 2][
                                :, (lqc % 2) * J * E + j * E:
                                (lqc % 2) * J * E + (j + 1) * E],
                            lhsT=et[:, lqc * P:(lqc + 1) * P],
                            rhs=va_list[j][:, ci, h * E:(h + 1) * E],
                            start=(j == 0 and ci == 0 and lqc % 2 == 0),
                            stop=(j == J - 1 and ci == 3 and lqc % 2 == 1))

                pend = []
                for j in range(J):
                    for ci in range(4):
                        pss = ps_att.tile([P, LQ], F32, tag="att", name="ps_s")
                        nc.tensor.matmul(
                            out=pss[:],
                            lhsT=kT[j][h // 2][:, ci * P:(ci + 1) * P],
                            rhs=qtp[h][:], start=True, stop=True)
                        et = expp.tile([P, LQ], BF16, tag="expp", name="et")
                        nc.scalar.activation(
                            out=et[:], in_=pss[:],
                            func=mybir.ActivationFunctionType.Exp,
                            scale=1.0 / math.sqrt(HD))
                        if dbg and b == 0 and h == 0 and j == 0:
                            nc.sync.dma_start(out=aps["dbg_exp"][ci], in_=et[:])
                        if fillers:
                            fillers.pop(0)()
                        pend.append((j, ci, et))
                        if len(pend) > 2:
                            emit_ctx(*pend.pop(0))
                while pend:
                    emit_ctx(*pend.pop(0))

                # normalize: denominator is column HD of each head stripe.
                # Each j-chunk goes into the head PAIR's shared catp tile
                # (64 columns per head) so the transposes and MLP1 run as
                # full 128x128 tiles with block-diagonal weights.
                if h % 2 == 0:
                    catp_tiles = [catp.tile([P, J * P], BF16, tag="catp",
                                            name="catp") for _ in range(4)]
                for pi in range(2):
                    rec = recp.tile([P, 2 * J], F32, tag="rec", name="rec")
                    nc.vector.reciprocal(
                        rec[:],
                        pscs[pi].rearrange("p (x e) -> p x e", e=E)[:, :, HD])
                    for half in range(2):
                        lqc = pi * 2 + half
                        # one fused multiply: [128, 3, 64] x per-(row,j)
                        # reciprocal broadcast along the last dim
                        nc.vector.tensor_tensor(
                            out=catp_tiles[lqc].rearrange(
                                "p (x e) -> p x e", e=P)[
                                :, :, (h % 2) * HD:(h % 2) * HD + HD],
                            in0=pscs[pi].rearrange(
                                "p (x e) -> p x e", e=E)[
                                :, half * J:(half + 1) * J, 0:HD],
                            in1=rec[:, half * J:(half + 1) * J].unsqueeze(2)
                            .to_broadcast((P, J, HD)),
                            op=mybir.AluOpType.mult)
                    if dbg and b == 0 and h == 0:
                        nc.sync.dma_start(out=aps["dbg_rec"][pi], in_=rec[:])
                if dbg and b == 0 and h == 1:
                    for lqc in range(4):
                        nc.sync.dma_start(out=aps["dbg_cat"][lqc],
                                          in_=catp_tiles[lqc][:, 0:P])

                # After the odd head, queue the pair's transpose work as PE
                # filler for the next head's exp-latency slots.
                if h % 2 == 1:
                    thunks = []
                    catTs = []
                    for j in range(J):
                        ptj = ps_big.tile([P, LQ], F32, tag="big", name="ptj")
                        catTj = ctp.tile([P, LQ], BF16, tag="ct", name="catTj")
                        for lqc in range(4):
                            thunks.append(
                                lambda lqc=lqc, ptj=ptj, j=j,
                                c=catp_tiles[lqc]:
                                nc.tensor.matmul(
                                    out=ptj[:, lqc * P:(lqc + 1) * P],
                                    lhsT=c[:, j * P:(j + 1) * P], rhs=ident[:],
                                    start=(lqc == 0), stop=(lqc == 3)))
                        thunks.append(
                            lambda ptj=ptj, catTj=catTj:
                            nc.vector.tensor_copy(out=catTj[:], in_=ptj[:]))
                        catTs.append(catTj)
                    catT_all[h // 2] = catTs
                    fillers.extend(thunks)
            while fillers:
                fillers.pop(0)()

            # ---- MLP1 / Gelu / MLP2 per head pair, full 128-tiles ----
            # tiny dummy activation pulls the Gelu table in while the PE is
            # still busy with transposes, instead of stalling the first gelu
            nc.scalar.activation(out=tbl_scr[:], in_=b1_sb[:],
                                 func=mybir.ActivationFunctionType.Gelu)
            for pi in range(hg * HG // 2, (hg + 1) * HG // 2):
                ph1p = ps_big.tile([P, LQ], F32, tag="big", name="ph1p")
                for j in range(J):
                    nc.tensor.matmul(out=ph1p[:, :], lhsT=w1jd[j][:],
                                     rhs=catT_all[pi][j][:],
                                     start=(j == 0), stop=(j == J - 1))
                h1 = h1p.tile([P, LQ], BF16, tag="h1", name="h1")
                nc.scalar.activation(
                    out=h1[:], in_=ph1p[:],
                    func=mybir.ActivationFunctionType.Gelu, bias=b1_sb[:])
                if dbg and b == 0 and pi == 0:
                    nc.sync.dma_start(out=aps["dbg_catT0"][:, :],
                                      in_=catT_all[0][0][:])
                    nc.sync.dma_start(out=aps["dbg_h1"][:, :], in_=h1[:])
                for lqc in range(4):
                    ps2 = ps_att.tile([P, LQ], F32, tag="att", name="ps2")
                    nc.tensor.matmul(
                        out=ps2[:, 0:P], lhsT=h1[:, lqc * P:(lqc + 1) * P],
                        rhs=w2bd[:], start=True, stop=True)
                    nc.vector.tensor_add(
                        ost[lqc][:, pi * P:(pi + 1) * P], ps2[:, 0:P],
                        b2_bc[:, pi * P:(pi + 1) * P])
            # preload the Exp table for the next attention phase
            nc.scalar.activation(out=tbl_scr[:], in_=b1_sb[:],
                                 func=mybir.ActivationFunctionType.Exp)
            # stream out this head-group's half of the output columns
            for lqc in range(4):
                nc.sync.dma_start(
                    out=aps["out"][b, lqc * P:(lqc + 1) * P,
                                   hg * 512:(hg + 1) * 512],
                    in_=ost[lqc][:, hg * 512:(hg + 1) * 512])


    for p in reversed(ctx_mgr):
        p.__exit__(None, None, None)


_CACHE = {}


def _build(dbg=False):
    key = ("nc", dbg)
    if key in _CACHE:
        return _CACHE[key]
    nc = bacc.Bacc("TRN2", target_bir_lowering=False, debug=False)
    shapes = {
        "qt_in": ([B_LOC, D, LQ], BF16),
        "kt_in": ([J, B_LOC, D, LK], BF16),
        "vt_in": ([J, B_LOC, D, LK], BF16),
        "wqt": ([D, D], BF16),
        "wkt": ([D, D], BF16),
        "wvt": ([D, D], BF16),
        "w1jd": ([J, P, P], BF16),
        "w2bd": ([P, P], BF16),
        "ident": ([P, P], BF16),
        "ones_cols": ([P, 4, H], BF16),
        "bq": ([P, 8], F32),
        "bk": ([P, 8], F32),
        "bv_bc": ([P, D], BF16),
        "b2_bc": ([P, D], F32),
        "b1": ([P, 1], F32),
    }
    aps = {k: nc.dram_tensor(k, s, dt, kind="ExternalInput").ap()
           for k, (s, dt) in shapes.items()}
    aps["out"] = nc.dram_tensor("out", [B_LOC, LQ, D], F32,
                                kind="ExternalOutput").ap()
    if dbg:
        dbg_shapes = {
            "dbg_qt": ([8, P, 512], BF16), "dbg_kt0": ([8, P, 512], BF16),
            "dbg_va0": ([P, 4, H * E], BF16), "dbg_exp": ([4, P, LQ], BF16),
            "dbg_rec": ([2, P, 2 * J], F32), "dbg_cat": ([4, P, P], BF16),
            "dbg_catT0": ([P, LQ], BF16), "dbg_h1": ([P, LQ], BF16),
        }
        for k, (shp, dt) in dbg_shapes.items():
            aps[k] = nc.dram_tensor(k, shp, dt, kind="ExternalOutput").ap()
    with tile.TileContext(nc) as tc:
        _emit(tc, aps, dbg=dbg)
    nc.compile()
    _CACHE[key] = nc
    return nc


def _prep_in_maps(inputs):
    f32 = np.float32
    bf16 = ml_dtypes.bfloat16
    q = np.ascontiguousarray(np.asarray(inputs["query_states"], f32))
    k = np.ascontiguousarray(np.asarray(inputs["key_states"], f32))
    v = np.ascontiguousarray(np.asarray(inputs["value_states"], f32))
    Wq = np.asarray(inputs["Wq"], f32)
    Wk = np.asarray(inputs["Wk"], f32)
    Wv = np.asarray(inputs["Wv"], f32)
    W1 = np.asarray(inputs["W1"], f32)
    W2 = np.asarray(inputs["W2"], f32)
    bq = np.asarray(inputs["bq"], f32)
    bk = np.asarray(inputs["bk"], f32)
    bv = np.asarray(inputs["bv"], f32)
    b1 = np.asarray(inputs["b1"], f32)
    b2 = np.asarray(inputs["b2"], f32)

    wqt = np.ascontiguousarray(Wq.T).astype(bf16)
    wkt = np.ascontiguousarray(Wk.T).astype(bf16)
    wvt = np.ascontiguousarray(Wv.T).astype(bf16)
    W1T = np.ascontiguousarray(W1.T)                       # [192, 64]
    w1jd = np.zeros((J, P, P), f32)
    for j in range(J):
        blk = W1T[j * HD:(j + 1) * HD]                     # [64, 64]
        w1jd[j, :HD, :HD] = blk
        w1jd[j, HD:, HD:] = blk
    w1jd = w1jd.astype(bf16)
    W2T = np.ascontiguousarray(W2.T)                       # [64, 64]
    w2bd = np.zeros((P, P), f32)
    w2bd[:HD, :HD] = W2T
    w2bd[HD:, HD:] = W2T
    w2bd = w2bd.astype(bf16)
    ident = np.eye(P, dtype=f32).astype(bf16)
    bq_sb = np.ascontiguousarray(bq.reshape(8, P).T).astype(f32)
    bk_sb = np.ascontiguousarray(bk.reshape(8, P).T).astype(f32)
    bv_bc = np.tile(bv, (P, 1)).astype(bf16)
    b2_bc = np.tile(b2, (P, H)).astype(f32)
    b1_col = np.concatenate([b1, b1]).reshape(P, 1).astype(f32)
    ones_cols = np.ones((P, 4, H), f32).astype(bf16)

    qt_all = np.ascontiguousarray(q.transpose(0, 2, 1)).astype(bf16)
    kt_all = np.ascontiguousarray(k.transpose(0, 1, 3, 2)).astype(bf16)
    vt_all = np.ascontiguousarray(v.transpose(0, 1, 3, 2)).astype(bf16)

    in_maps = []
    for c in range(N_CORES):
        sl = slice(c * B_LOC, (c + 1) * B_LOC)
        in_maps.append({
            "qt_in": np.ascontiguousarray(qt_all[sl]),
            "kt_in": np.ascontiguousarray(kt_all[:, sl]),
            "vt_in": np.ascontiguousarray(vt_all[:, sl]),
            "wqt": wqt, "wkt": wkt, "wvt": wvt,
            "w1jd": w1jd, "w2bd": w2bd, "ident": ident,
            "ones_cols": ones_cols,
            "bq": bq_sb, "bk": bk_sb, "bv_bc": bv_bc,
            "b2_bc": b2_bc, "b1": b1_col,
        })
    return in_maps


def kernel(**inputs):
    nc = _build()
    in_maps = _prep_in_maps(inputs)
    res = run_bass_kernel_spmd(nc, in_maps, core_ids=list(range(N_CORES)))
    out = np.concatenate([res.results[i]["out"] for i in range(N_CORES)], axis=0)
    return out.astype(np.float32)
